# revision 18
# baseline (speedup 1.0000x reference)
"""GCN encoder (2-layer, BN, residual) on 8 Trainium2 NeuronCores.

Sharding: nodes partitioned contiguously across 8 cores (6250 each). Edges
bucketed by dst shard on host (integer-only preprocessing: bucket/sort/pad
edge indices, degree counts via bincount). All float math runs on device:

  - per-node norm d_out=rsqrt(clip(outdeg,1)) folded into an fp16 copy of the
    gather table (x*d_out, AllGathered to every core)
  - SpMM: dma_gather of 128-edge chunks (rows->partitions) + one-hot selector
    matmul on PE accumulating m^T[feat, dst] in PSUM; selector built on DVE
    from iota==slot compare (exact 0/1 entries)
  - d_in applied via a broadcast matrix during PSUM evacuation
  - W matmul with W as the stationary operand keeps the [feat, dst] layout so
    BN (per-feature affine) uses per-partition ACT scale/bias + fused ReLU
  - BN stats: per-core partial sums + 1KB AllReduce
  - layer-2 table: h1*d_out cast fp16, AllGathered
"""

import sys

sys.path.insert(0, "/opt/trn_rl_repo")

import numpy as np

P = 128
N_CORES = 8
EPS = 1e-5

# compute dtype for gather tables / selectors / segment matmul
_F16 = "float16"

# dma_gather tuning (device crashes observed for very large single calls)
GATHER_SINGLE_PACKET = True
GATHER_MAX_IDX = 768  # max indices per dma_gather instruction (larger crashes device)
DMA_SCRATCH = 65536  # per-partition SWDGE descriptor-ring carveout


def _cdiv(a, b):
    return -(-a // b)


# ---------------------------------------------------------------------------
# host-side integer preprocessing (indices only; no float arithmetic on data)
# ---------------------------------------------------------------------------


def _wrap_idx_image(idx_list):
    """int16 index list (len % 16 == 0) -> [128, len/16] SBUF image.

    dma_gather reads idx i from partition i%16, free slot i//16; the 16-row
    pattern must be replicated 8x across the 128 partitions (one per Q7 core).
    """
    n = idx_list.shape[0]
    assert n % 16 == 0
    img16 = idx_list.reshape(n // 16, 16).T  # [16, n/16]
    return np.tile(img16, (8, 1)).astype(np.int16)  # [128, n/16]


def _host_prep(src, dst, n_nodes):
    """Bucket edges by (dst shard, dst tile, src half); pad to uniform chunk
    capacities so all 8 cores run one identical program."""
    NC = N_CORES
    SH = n_nodes // NC
    assert SH * NC == n_nodes
    T = _cdiv(SH, P)
    SPLIT = n_nodes // 2
    assert SPLIT < 32768 and (n_nodes - SPLIT) <= 32768

    src = np.asarray(src, np.int64)
    dst = np.asarray(dst, np.int64)

    per_core = []
    CA = CB = 1
    for k in range(NC):
        m = (dst >= k * SH) & (dst < (k + 1) * SH)
        s = src[m]
        dl = dst[m] - k * SH
        t_idx = dl // P
        slot = dl % P
        half = (s >= SPLIT).astype(np.int64)
        idxval = np.where(half == 1, s - SPLIT, s)
        per_core.append((t_idx, half, idxval, slot))
        for t in range(T):
            tm = t_idx == t
            na = int(np.count_nonzero(tm & (half == 0)))
            nb = int(np.count_nonzero(tm & (half == 1)))
            CA = max(CA, _cdiv(na, P))
            CB = max(CB, _cdiv(nb, P))

    n_chunks = T * (CA + CB)
    pairs = [(2 * b, min(2 * b + 1, T - 1)) for b in range(_cdiv(T, 2))]

    cores = []
    for k in range(NC):
        t_idx, half, idxval, slot = per_core[k]
        A_idx = np.zeros((T, CA * P), np.int16)
        B_idx = np.zeros((T, CB * P), np.int16)
        # pad slot = 255: one-hot (iota==slot) never fires -> zero column
        slots = np.full((n_chunks, P), 255.0, np.float16)
        for t in range(T):
            tm = t_idx == t
            for h, (Cc, buf) in enumerate(((CA, A_idx), (CB, B_idx))):
                hm = tm & (half == h)
                iv = idxval[hm]
                sl = slot[hm]
                n = iv.shape[0]
                buf[t, :n] = iv.astype(np.int16)
                base = t * (CA + CB) + (0 if h == 0 else CA)
                for c in range(Cc):
                    lo, hi = c * P, min((c + 1) * P, n)
                    if hi > lo:
                        slots[base + c, : hi - lo] = sl[lo:hi].astype(np.float16)

        # gather-call index images: one A call + one B call per tile pair
        imgs = []
        offs_a, offs_b = [], []
        col = 0
        for t0, t1 in pairs:
            tl = [t0] if t0 == t1 else [t0, t1]
            for h, (Cc, buf, offs) in enumerate(
                ((CA, A_idx, offs_a), (CB, B_idx, offs_b))
            ):
                lst = np.concatenate([buf[t] for t in tl])
                img = _wrap_idx_image(lst)
                offs.append((col, img.shape[1], len(tl) * Cc * P))
                col += img.shape[1]
                imgs.append(img)
        idx_img = np.concatenate(imgs, axis=1)  # [128, col]

        # degree counts (integers), tile-column layout [P, T], pad rows deg=1
        outdeg = np.bincount(src, minlength=n_nodes).astype(np.int64)
        indeg = np.bincount(dst, minlength=n_nodes).astype(np.int64)
        mine = slice(k * SH, (k + 1) * SH)

        def _cols(d):
            v = np.ones(T * P, np.float32)
            v[:SH] = d[mine].astype(np.float32)
            return v.reshape(T, P).T.copy()  # [P, T]

        cores.append(
            dict(
                idx_img=idx_img,
                slotT=slots.T.copy(),  # [P, n_chunks] fp16
                deg_out=_cols(outdeg),
                deg_in=_cols(indeg),
                offs_a=offs_a,
                offs_b=offs_b,
            )
        )

    meta = dict(
        SH=SH,
        T=T,
        SPLIT=SPLIT,
        CA=CA,
        CB=CB,
        n_chunks=n_chunks,
        pairs=pairs,
        idx_cols=cores[0]["idx_img"].shape[1],
        n_nodes=n_nodes,
        # call offsets are identical across cores by construction
        offs_a=cores[0]["offs_a"],
        offs_b=cores[0]["offs_b"],
    )
    for c in cores[1:]:
        assert c["offs_a"] == meta["offs_a"] and c["offs_b"] == meta["offs_b"]
        assert c["idx_img"].shape == cores[0]["idx_img"].shape
    return meta, cores


# ---------------------------------------------------------------------------
# device program (identical on all cores; all data-dependence through SBUF)
# ---------------------------------------------------------------------------


def _build_program(meta):
    import concourse.bacc as bacc
    import concourse.bass as bass
    import concourse.tile as tile
    from concourse import mybir
    from concourse.masks import make_identity

    f32 = mybir.dt.float32
    f16 = getattr(mybir.dt, _F16)
    Alu = mybir.AluOpType
    Act = mybir.ActivationFunctionType

    SH, T, SPLIT = meta["SH"], meta["T"], meta["SPLIT"]
    CA, CB = meta["CA"], meta["CB"]
    NCH = meta["n_chunks"]
    NN = meta["n_nodes"]
    pairs = meta["pairs"]
    rows_of = lambda t: min(P, SH - t * P)

    nc = bacc.Bacc(
        "TRN2",
        target_bir_lowering=False,
        debug=False,
        num_devices=N_CORES,
        dynamic_dma_scratch_size=DMA_SCRATCH,
    )

    # ---- I/O -------------------------------------------------------------
    x_shard = nc.dram_tensor("x_shard", [SH, P], f32, kind="ExternalInput")
    W1_t = nc.dram_tensor("W1", [P, P], f32, kind="ExternalInput")
    W2_t = nc.dram_tensor("W2", [P, P], f32, kind="ExternalInput")
    gm1 = nc.dram_tensor("gamma1", [P, 1], f32, kind="ExternalInput")
    bt1 = nc.dram_tensor("beta1", [P, 1], f32, kind="ExternalInput")
    gm2 = nc.dram_tensor("gamma2", [P, 1], f32, kind="ExternalInput")
    bt2 = nc.dram_tensor("beta2", [P, 1], f32, kind="ExternalInput")
    iota_t = nc.dram_tensor("iota", [P, P], f16, kind="ExternalInput")
    idx_t = nc.dram_tensor("idx_img", [P, meta["idx_cols"]], mybir.dt.int16,
                           kind="ExternalInput")
    slot_t = nc.dram_tensor("slotT", [P, NCH], f16, kind="ExternalInput")
    dego_t = nc.dram_tensor("deg_out", [P, T], f32, kind="ExternalInput")
    degi_t = nc.dram_tensor("deg_in", [P, T], f32, kind="ExternalInput")
    out_t = nc.dram_tensor("out", [SH, P], f32, kind="ExternalOutput")

    with tile.TileContext(nc) as tc:
        with (
            tc.tile_pool(name="cst", bufs=1) as cst,
            tc.tile_pool(name="big", bufs=1) as big,
            tc.tile_pool(name="gat", bufs=2) as gat,
            tc.tile_pool(name="wrk", bufs=3) as wrk,
            tc.tile_pool(name="ps", bufs=2, space="PSUM") as ps,
            tc.tile_pool(name="dram", bufs=1, space="DRAM") as dram,
        ):
            # ---- constants / static data --------------------------------
            ident = cst.tile([P, P], f32)
            make_identity(nc, ident[:])
            W1s = cst.tile([P, P], f32)
            W2s = cst.tile([P, P], f32)
            iota = cst.tile([P, P], f16)
            nc.sync.dma_start(W1s[:], W1_t[:])
            nc.sync.dma_start(W2s[:], W2_t[:])
            nc.sync.dma_start(iota[:], iota_t[:])
            idx_sb = cst.tile([P, meta["idx_cols"]], mybir.dt.int16)
            nc.sync.dma_start(idx_sb[:], idx_t[:])
            slot_sb = cst.tile([P, NCH], f16)
            nc.sync.dma_start(slot_sb[:], slot_t[:])
            gm1s = cst.tile([P, 1], f32)
            bt1s = cst.tile([P, 1], f32)
            gm2s = cst.tile([P, 1], f32)
            bt2s = cst.tile([P, 1], f32)
            nc.sync.dma_start(gm1s[:], gm1[:])
            nc.sync.dma_start(bt1s[:], bt1[:])
            nc.sync.dma_start(gm2s[:], gm2[:])
            nc.sync.dma_start(bt2s[:], bt2[:])

            # ---- degree normalizers (float math on device) --------------
            d_out = cst.tile([P, T], f32)
            d_in = cst.tile([P, T], f32)
            for deg_dram, d_sb in ((dego_t, d_out), (degi_t, d_in)):
                raw = wrk.tile([P, T], f32, tag="degraw")
                nc.sync.dma_start(raw[:], deg_dram[:])
                nc.vector.tensor_scalar_max(raw[:], raw[:], 1.0)
                nc.scalar.sqrt(raw[:], raw[:])
                nc.vector.reciprocal(d_sb[:], raw[:])

            # d_in broadcast rows: din_bc[:, t*P+j] = d_in[j, t] for all rows
            din_bc = big.tile([P, T * P], f32)
            for t in range(T):
                bc_ps = ps.tile([P, P], f32, tag="tp")
                nc.tensor.transpose(
                    out=bc_ps[:],
                    in_=d_in[:, t : t + 1].to_broadcast([P, P]),
                    identity=ident[:],
                )
                nc.vector.tensor_copy(din_bc[:, t * P : (t + 1) * P], bc_ps[:])

            # ---- fp16 normalized gather table for layer 1 ---------------
            x16_shard = dram.tile([SH, P], f16)
            x16_full = dram.tile([NN, P], f16, addr_space="Shared")
            for t in range(T):
                r = rows_of(t)
                xt = wrk.tile([P, P], f32, tag="xload")
                nc.sync.dma_start(xt[:r, :], x_shard[t * P : t * P + r, :])
                st = wrk.tile([P, P], f16, tag="xstage")
                nc.vector.tensor_scalar(
                    st[:r, :], xt[:r, :], d_out[:r, t : t + 1], None, Alu.mult
                )
                nc.sync.dma_start(x16_shard[t * P : t * P + r, :], st[:r, :])
            nc.gpsimd.collective_compute(
                "AllGather",
                Alu.bypass,
                replica_groups=[list(range(N_CORES))],
                ins=[x16_shard.opt()],
                outs=[x16_full.opt()],
            )

            # persistent stores
            hpre = big.tile([P, T * P], f32)   # pre-BN activations [feat, dst]
            h1 = big.tile([P, T * P], f32)     # post-BN/relu layer-1 output
            h16_shard = dram.tile([SH, P], f16)
            h16_full = dram.tile([NN, P], f16, addr_space="Shared")

            def gconv_layer(table_full, W_sb, s1_cols, s2_cols):
                """SpMM + W matmul; fills hpre and the per-tile stat columns."""
                srcA = table_full[0:SPLIT, :]
                srcB = table_full[SPLIT:NN, :]
                for ip, (t0, t1) in enumerate(pairs):
                    tl = [t0] if t0 == t1 else [t0, t1]
                    bufs = {}
                    for h, (Cc, offs, sv) in enumerate(
                        ((CA, meta["offs_a"], srcA), (CB, meta["offs_b"], srcB))
                    ):
                        col, wcols, nidx = offs[ip]
                        g = gat.tile([P, 2 * Cc, P], f16, tag=f"g{h}")
                        nch = nidx // P
                        step = max(1, GATHER_MAX_IDX // P)
                        for c0 in range(0, nch, step):
                            c1 = min(c0 + step, nch)
                            nc.gpsimd.dma_gather(
                                g[:, c0:c1, :],
                                sv,
                                idx_sb[:, col + c0 * 8 : col + c1 * 8],
                                (c1 - c0) * P,
                                (c1 - c0) * P,
                                P,
                                single_packet=GATHER_SINGLE_PACKET,
                            )
                        bufs[h] = g
                    for ti, t in enumerate(tl):
                        mT = ps.tile([P, P], f32, tag="mT")
                        cid0 = t * (CA + CB)
                        NCHT = CA + CB
                        # one-hot selectors for the tile's chunks in one DVE op
                        sel = wrk.tile([P, NCHT, P], f16, tag="sel", bufs=3)
                        nc.vector.tensor_tensor(
                            out=sel[:],
                            in0=slot_sb[:, cid0 : cid0 + NCHT][:, :, None]
                            .to_broadcast([P, NCHT, P]),
                            in1=iota[:, None, :].to_broadcast([P, NCHT, P]),
                            op=Alu.is_equal,
                        )
                        for c in range(NCHT):
                            h, cc = (0, c) if c < CA else (1, c - CA)
                            Cc = CA if h == 0 else CB
                            nc.tensor.matmul(
                                out=mT[:],
                                lhsT=bufs[h][:, ti * Cc + cc, :],
                                rhs=sel[:, c, :],
                                start=(c == 0),
                                stop=(c == NCHT - 1),
                            )
                        # evacuate with d_in column scaling
                        mTs = wrk.tile([P, P], f32, tag="mTs")
                        nc.vector.tensor_tensor(
                            out=mTs[:],
                            in0=mT[:],
                            in1=din_bc[:, t * P : (t + 1) * P],
                            op=Alu.mult,
                        )
                        hp = ps.tile([P, P], f32, tag="hp")
                        nc.tensor.matmul(
                            out=hp[:], lhsT=W_sb[:], rhs=mTs[:], start=True, stop=True
                        )
                        # evacuate + per-feature partial sums for BN
                        nc.vector.tensor_scalar(
                            hpre[:, t * P : (t + 1) * P],
                            hp[:],
                            1.0,
                            None,
                            Alu.mult,
                            Alu.add,
                            accum_out=s1_cols[:, t : t + 1],
                        )
                        sq = wrk.tile([P, P], f16, tag="sq")
                        nc.scalar.activation(
                            sq[:],
                            hpre[:, t * P : (t + 1) * P],
                            Act.Square,
                            accum_out=s2_cols[:, t : t + 1],
                        )

            def bn_coeffs(s1_cols, s2_cols, gam, bet, tag):
                """AllReduce partial sums -> per-feature scale a, shift c."""
                stats_in = dram.tile([P, 2], f32, name=f"stats_in_{tag}")
                stats_out = dram.tile(
                    [P, 2], f32, addr_space="Shared", name=f"stats_out_{tag}"
                )
                pack = wrk.tile([P, 2], f32, tag="pack")
                nc.vector.tensor_reduce(
                    pack[:, 0:1], s1_cols[:], axis=mybir.AxisListType.X, op=Alu.add
                )
                nc.vector.tensor_reduce(
                    pack[:, 1:2], s2_cols[:], axis=mybir.AxisListType.X, op=Alu.add
                )
                nc.sync.dma_start(stats_in[:], pack[:])
                nc.gpsimd.collective_compute(
                    "AllReduce",
                    Alu.add,
                    replica_groups=[list(range(N_CORES))],
                    ins=[stats_in.opt()],
                    outs=[stats_out.opt()],
                )
                glob = wrk.tile([P, 2], f32, tag="glob")
                nc.sync.dma_start(glob[:], stats_out[:])
                mo = wrk.tile([P, 4], f32, tag="mo")
                # mo: 0=mu 1=E[h^2] 2=var+eps 3=scratch
                nc.vector.tensor_scalar(mo[:, 0:2], glob[:], 1.0 / NN, None, Alu.mult)
                nc.vector.tensor_tensor(
                    out=mo[:, 3:4], in0=mo[:, 0:1], in1=mo[:, 0:1], op=Alu.mult
                )
                nc.vector.tensor_tensor(
                    out=mo[:, 2:3], in0=mo[:, 1:2], in1=mo[:, 3:4], op=Alu.subtract
                )
                nc.vector.tensor_scalar_add(mo[:, 2:3], mo[:, 2:3], EPS)
                nc.scalar.sqrt(mo[:, 2:3], mo[:, 2:3])
                a_c = cst.tile([P, 2], f32, name=f"a_c_{gam.name}")
                nc.vector.reciprocal(a_c[:, 0:1], mo[:, 2:3])
                nc.vector.tensor_tensor(
                    out=a_c[:, 0:1], in0=a_c[:, 0:1], in1=gam[:], op=Alu.mult
                )
                nc.vector.tensor_tensor(
                    out=a_c[:, 1:2], in0=a_c[:, 0:1], in1=mo[:, 0:1], op=Alu.mult
                )
                nc.vector.tensor_tensor(
                    out=a_c[:, 1:2], in0=bet[:], in1=a_c[:, 1:2], op=Alu.subtract
                )
                return a_c

            # ================= layer 1 =================
            s1a = cst.tile([P, T], f32)
            s2a = cst.tile([P, T], f32)
            gconv_layer(x16_full, W1s, s1a, s2a)
            ac1 = bn_coeffs(s1a, s2a, gm1s, bt1s, "l1")

            # BN + relu -> h1; build fp16 normalized layer-2 table
            for t in range(T):
                r = rows_of(t)
                nc.scalar.activation(
                    h1[:, t * P : (t + 1) * P],
                    hpre[:, t * P : (t + 1) * P],
                    Act.Relu,
                    bias=ac1[:, 1:2],
                    scale=ac1[:, 0:1],
                )
                tp = ps.tile([P, P], f32, tag="tp")
                nc.tensor.transpose(
                    out=tp[:], in_=h1[:, t * P : (t + 1) * P], identity=ident[:]
                )
                st = wrk.tile([P, P], f16, tag="xstage")
                nc.vector.tensor_scalar(
                    st[:r, :], tp[:r, :], d_out[:r, t : t + 1], None, Alu.mult
                )
                nc.sync.dma_start(h16_shard[t * P : t * P + r, :], st[:r, :])
            nc.gpsimd.collective_compute(
                "AllGather",
                Alu.bypass,
                replica_groups=[list(range(N_CORES))],
                ins=[h16_shard.opt()],
                outs=[h16_full.opt()],
            )

            # ================= layer 2 =================
            s1b = cst.tile([P, T], f32)
            s2b = cst.tile([P, T], f32)
            gconv_layer(h16_full, W2s, s1b, s2b)
            ac2 = bn_coeffs(s1b, s2b, gm2s, bt2s, "l2")

            for t in range(T):
                r = rows_of(t)
                h2 = wrk.tile([P, P], f32, tag="h2")
                nc.scalar.activation(
                    h2[:],
                    hpre[:, t * P : (t + 1) * P],
                    Act.Identity,
                    bias=ac2[:, 1:2],
                    scale=ac2[:, 0:1],
                )
                nc.vector.tensor_tensor(
                    out=h2[:], in0=h2[:], in1=h1[:, t * P : (t + 1) * P], op=Alu.add
                )
                nc.scalar.activation(h2[:], h2[:], Act.Relu)
                tp = ps.tile([P, P], f32, tag="tp")
                nc.tensor.transpose(out=tp[:], in_=h2[:], identity=ident[:])
                ot = wrk.tile([P, P], f32, tag="ostage")
                nc.vector.tensor_copy(ot[:r, :], tp[:r, :])
                nc.sync.dma_start(out_t[t * P : t * P + r, :], ot[:r, :])

    nc.compile()
    return nc


# ---------------------------------------------------------------------------


_CACHE = {}


def _get_program(meta):
    key = (meta["SH"], meta["T"], meta["CA"], meta["CB"], meta["idx_cols"])
    if key not in _CACHE:
        _CACHE[key] = _build_program(meta)
    return _CACHE[key]


def kernel(**inputs):
    x = np.asarray(inputs["x"], np.float32)
    src = np.asarray(inputs["src"])
    dst = np.asarray(inputs["dst"])
    n_nodes = x.shape[0]

    meta, cores = _host_prep(src, dst, n_nodes)
    nc = _get_program(meta)

    SH = meta["SH"]
    iota = np.tile(np.arange(P, dtype=np.float16), (P, 1))
    in_maps = []
    for k in range(N_CORES):
        c = cores[k]
        in_maps.append(
            {
                "x_shard": np.ascontiguousarray(x[k * SH : (k + 1) * SH]),
                "W1": np.asarray(inputs["W1"], np.float32),
                "W2": np.asarray(inputs["W2"], np.float32),
                "gamma1": np.asarray(inputs["gamma1"], np.float32).reshape(P, 1),
                "beta1": np.asarray(inputs["beta1"], np.float32).reshape(P, 1),
                "gamma2": np.asarray(inputs["gamma2"], np.float32).reshape(P, 1),
                "beta2": np.asarray(inputs["beta2"], np.float32).reshape(P, 1),
                "iota": iota,
                "idx_img": c["idx_img"],
                "slotT": c["slotT"],
                "deg_out": c["deg_out"],
                "deg_in": c["deg_in"],
            }
        )

    from concourse.bass_utils import run_bass_kernel_spmd

    res = run_bass_kernel_spmd(nc, in_maps, core_ids=list(range(N_CORES)))
    out = np.concatenate([res.results[k]["out"] for k in range(N_CORES)], axis=0)
    return out.astype(np.float32)


# revision 22
# speedup vs baseline: 2.1948x; 2.1948x over previous
"""GCN encoder (2-layer, BN, residual) on 8 Trainium2 NeuronCores.

Sharding: nodes partitioned contiguously across 8 cores (6250 each). Edges
bucketed by dst shard on host (integer-only preprocessing: bucket/sort/pad
edge indices, degree counts via bincount). All float math runs on device:

  - per-node norm d_out=rsqrt(clip(outdeg,1)) folded into an fp16 copy of the
    gather table (x*d_out, AllGathered to every core)
  - SpMM: dma_gather of 128-edge chunks (rows->partitions) + one-hot selector
    matmul on PE accumulating m^T[feat, dst] in PSUM; selector built on DVE
    from iota==slot compare (exact 0/1 entries)
  - d_in applied via a broadcast matrix during PSUM evacuation
  - W matmul with W as the stationary operand keeps the [feat, dst] layout so
    BN (per-feature affine) uses per-partition ACT scale/bias + fused ReLU
  - BN stats: per-core partial sums + 1KB AllReduce
  - layer-2 table: h1*d_out cast fp16, AllGathered
"""

import sys

sys.path.insert(0, "/opt/trn_rl_repo")

import numpy as np

P = 128
N_CORES = 8
EPS = 1e-5

# compute dtype for gather tables / selectors / segment matmul
_F16 = "float16"

# dma_gather tuning (device crashes observed for very large single calls)
GATHER_SINGLE_PACKET = False
GATHER_MAX_IDX = 768  # max indices per dma_gather instruction (larger crashes device)
DMA_SCRATCH = 65536  # per-partition SWDGE descriptor-ring carveout
N_SWDGE_QUEUES = 4  # each queue runs on its own Q7 core pair -> parallel desc-gen


def _cdiv(a, b):
    return -(-a // b)


# ---------------------------------------------------------------------------
# host-side integer preprocessing (indices only; no float arithmetic on data)
# ---------------------------------------------------------------------------


def _wrap_idx_image(idx_list):
    """int16 index list (len % 16 == 0) -> [128, len/16] SBUF image.

    dma_gather reads idx i from partition i%16, free slot i//16; the 16-row
    pattern must be replicated 8x across the 128 partitions (one per Q7 core).
    """
    n = idx_list.shape[0]
    assert n % 16 == 0
    img16 = idx_list.reshape(n // 16, 16).T  # [16, n/16]
    return np.tile(img16, (8, 1)).astype(np.int16)  # [128, n/16]


def _host_prep(src, dst, n_nodes):
    """Bucket edges by (dst shard, dst tile, src half); pad to uniform chunk
    capacities so all 8 cores run one identical program."""
    NC = N_CORES
    SH = n_nodes // NC
    assert SH * NC == n_nodes
    T = _cdiv(SH, P)
    SPLIT = n_nodes // 2
    assert SPLIT < 32768 and (n_nodes - SPLIT) <= 32768

    src = np.asarray(src, np.int64)
    dst = np.asarray(dst, np.int64)

    per_core = []
    CA = CB = 1
    for k in range(NC):
        m = (dst >= k * SH) & (dst < (k + 1) * SH)
        s = src[m]
        dl = dst[m] - k * SH
        t_idx = dl // P
        slot = dl % P
        half = (s >= SPLIT).astype(np.int64)
        idxval = np.where(half == 1, s - SPLIT, s)
        per_core.append((t_idx, half, idxval, slot))
        for t in range(T):
            tm = t_idx == t
            na = int(np.count_nonzero(tm & (half == 0)))
            nb = int(np.count_nonzero(tm & (half == 1)))
            CA = max(CA, _cdiv(na, P))
            CB = max(CB, _cdiv(nb, P))

    n_chunks = T * (CA + CB)
    pairs = [(2 * b, min(2 * b + 1, T - 1)) for b in range(_cdiv(T, 2))]

    cores = []
    for k in range(NC):
        t_idx, half, idxval, slot = per_core[k]
        A_idx = np.zeros((T, CA * P), np.int16)
        B_idx = np.zeros((T, CB * P), np.int16)
        # pad slot = 255: one-hot (iota==slot) never fires -> zero column
        slots = np.full((n_chunks, P), 255.0, np.float16)
        for t in range(T):
            tm = t_idx == t
            for h, (Cc, buf) in enumerate(((CA, A_idx), (CB, B_idx))):
                hm = tm & (half == h)
                iv = idxval[hm]
                sl = slot[hm]
                n = iv.shape[0]
                buf[t, :n] = iv.astype(np.int16)
                base = t * (CA + CB) + (0 if h == 0 else CA)
                for c in range(Cc):
                    lo, hi = c * P, min((c + 1) * P, n)
                    if hi > lo:
                        slots[base + c, : hi - lo] = sl[lo:hi].astype(np.float16)

        # gather-call index images: one A call + one B call per tile pair
        imgs = []
        offs_a, offs_b = [], []
        col = 0
        for t0, t1 in pairs:
            tl = [t0] if t0 == t1 else [t0, t1]
            for h, (Cc, buf, offs) in enumerate(
                ((CA, A_idx, offs_a), (CB, B_idx, offs_b))
            ):
                lst = np.concatenate([buf[t] for t in tl])
                img = _wrap_idx_image(lst)
                offs.append((col, img.shape[1], len(tl) * Cc * P))
                col += img.shape[1]
                imgs.append(img)
        idx_img = np.concatenate(imgs, axis=1)  # [128, col]

        # degree counts (integers), tile-column layout [P, T], pad rows deg=1
        outdeg = np.bincount(src, minlength=n_nodes).astype(np.int64)
        indeg = np.bincount(dst, minlength=n_nodes).astype(np.int64)
        mine = slice(k * SH, (k + 1) * SH)

        def _cols(d):
            v = np.ones(T * P, np.float32)
            v[:SH] = d[mine].astype(np.float32)
            return v.reshape(T, P).T.copy()  # [P, T]

        cores.append(
            dict(
                idx_img=idx_img,
                slotT=slots.T.copy(),  # [P, n_chunks] fp16
                deg_out=_cols(outdeg),
                deg_in=_cols(indeg),
                offs_a=offs_a,
                offs_b=offs_b,
            )
        )

    meta = dict(
        SH=SH,
        T=T,
        SPLIT=SPLIT,
        CA=CA,
        CB=CB,
        n_chunks=n_chunks,
        pairs=pairs,
        idx_cols=cores[0]["idx_img"].shape[1],
        n_nodes=n_nodes,
        # call offsets are identical across cores by construction
        offs_a=cores[0]["offs_a"],
        offs_b=cores[0]["offs_b"],
    )
    for c in cores[1:]:
        assert c["offs_a"] == meta["offs_a"] and c["offs_b"] == meta["offs_b"]
        assert c["idx_img"].shape == cores[0]["idx_img"].shape
    return meta, cores


# ---------------------------------------------------------------------------
# device program (identical on all cores; all data-dependence through SBUF)
# ---------------------------------------------------------------------------


def _build_program(meta):
    import concourse.bacc as bacc
    import concourse.bass as bass
    import concourse.tile as tile
    from concourse import mybir
    from concourse.masks import make_identity

    f32 = mybir.dt.float32
    f16 = getattr(mybir.dt, _F16)
    Alu = mybir.AluOpType
    Act = mybir.ActivationFunctionType

    SH, T, SPLIT = meta["SH"], meta["T"], meta["SPLIT"]
    CA, CB = meta["CA"], meta["CB"]
    NCH = meta["n_chunks"]
    NN = meta["n_nodes"]
    pairs = meta["pairs"]
    rows_of = lambda t: min(P, SH - t * P)

    nc = bacc.Bacc(
        "TRN2",
        target_bir_lowering=False,
        debug=False,
        num_devices=N_CORES,
        dynamic_dma_scratch_size=DMA_SCRATCH,
        num_swdge_queues=N_SWDGE_QUEUES,
    )

    # ---- I/O -------------------------------------------------------------
    x_shard = nc.dram_tensor("x_shard", [SH, P], f32, kind="ExternalInput")
    W1_t = nc.dram_tensor("W1", [P, P], f32, kind="ExternalInput")
    W2_t = nc.dram_tensor("W2", [P, P], f32, kind="ExternalInput")
    gm1 = nc.dram_tensor("gamma1", [P, 1], f32, kind="ExternalInput")
    bt1 = nc.dram_tensor("beta1", [P, 1], f32, kind="ExternalInput")
    gm2 = nc.dram_tensor("gamma2", [P, 1], f32, kind="ExternalInput")
    bt2 = nc.dram_tensor("beta2", [P, 1], f32, kind="ExternalInput")
    iota_t = nc.dram_tensor("iota", [P, P], f16, kind="ExternalInput")
    idx_t = nc.dram_tensor("idx_img", [P, meta["idx_cols"]], mybir.dt.int16,
                           kind="ExternalInput")
    slot_t = nc.dram_tensor("slotT", [P, NCH], f16, kind="ExternalInput")
    dego_t = nc.dram_tensor("deg_out", [P, T], f32, kind="ExternalInput")
    degi_t = nc.dram_tensor("deg_in", [P, T], f32, kind="ExternalInput")
    out_t = nc.dram_tensor("out", [SH, P], f32, kind="ExternalOutput")

    with tile.TileContext(nc) as tc:
        with (
            tc.tile_pool(name="cst", bufs=1) as cst,
            tc.tile_pool(name="big", bufs=1) as big,
            tc.tile_pool(name="gat", bufs=2) as gat,
            tc.tile_pool(name="wrk", bufs=3) as wrk,
            tc.tile_pool(name="ps", bufs=2, space="PSUM") as ps,
            tc.tile_pool(name="dram", bufs=1, space="DRAM") as dram,
        ):
            # ---- constants / static data --------------------------------
            ident = cst.tile([P, P], f32)
            make_identity(nc, ident[:])
            W1s = cst.tile([P, P], f32)
            W2s = cst.tile([P, P], f32)
            iota = cst.tile([P, P], f16)
            nc.sync.dma_start(W1s[:], W1_t[:])
            nc.sync.dma_start(W2s[:], W2_t[:])
            nc.sync.dma_start(iota[:], iota_t[:])
            idx_sb = cst.tile([P, meta["idx_cols"]], mybir.dt.int16)
            nc.sync.dma_start(idx_sb[:], idx_t[:])
            slot_sb = cst.tile([P, NCH], f16)
            nc.sync.dma_start(slot_sb[:], slot_t[:])
            gm1s = cst.tile([P, 1], f32)
            bt1s = cst.tile([P, 1], f32)
            gm2s = cst.tile([P, 1], f32)
            bt2s = cst.tile([P, 1], f32)
            nc.sync.dma_start(gm1s[:], gm1[:])
            nc.sync.dma_start(bt1s[:], bt1[:])
            nc.sync.dma_start(gm2s[:], gm2[:])
            nc.sync.dma_start(bt2s[:], bt2[:])

            # ---- degree normalizers (float math on device) --------------
            d_out = cst.tile([P, T], f32)
            d_in = cst.tile([P, T], f32)
            for deg_dram, d_sb in ((dego_t, d_out), (degi_t, d_in)):
                raw = wrk.tile([P, T], f32, tag="degraw")
                nc.sync.dma_start(raw[:], deg_dram[:])
                nc.vector.tensor_scalar_max(raw[:], raw[:], 1.0)
                nc.scalar.sqrt(raw[:], raw[:])
                nc.vector.reciprocal(d_sb[:], raw[:])

            # d_in broadcast rows: din_bc[:, t*P+j] = d_in[j, t] for all rows
            din_bc = big.tile([P, T * P], f32)
            for t in range(T):
                bc_ps = ps.tile([P, P], f32, tag="tp")
                nc.tensor.transpose(
                    out=bc_ps[:],
                    in_=d_in[:, t : t + 1].to_broadcast([P, P]),
                    identity=ident[:],
                )
                nc.vector.tensor_copy(din_bc[:, t * P : (t + 1) * P], bc_ps[:])

            # ---- fp16 normalized gather table for layer 1 ---------------
            x16_shard = dram.tile([SH, P], f16)
            x16_full = dram.tile([NN, P], f16, addr_space="Shared")
            for t in range(T):
                r = rows_of(t)
                xt = wrk.tile([P, P], f32, tag="xload")
                nc.sync.dma_start(xt[:r, :], x_shard[t * P : t * P + r, :])
                st = wrk.tile([P, P], f16, tag="xstage")
                nc.vector.tensor_scalar(
                    st[:r, :], xt[:r, :], d_out[:r, t : t + 1], None, Alu.mult
                )
                nc.sync.dma_start(x16_shard[t * P : t * P + r, :], st[:r, :])
            nc.gpsimd.collective_compute(
                "AllGather",
                Alu.bypass,
                replica_groups=[list(range(N_CORES))],
                ins=[x16_shard.opt()],
                outs=[x16_full.opt()],
            )

            # persistent stores
            hpre = big.tile([P, T * P], f32)   # pre-BN activations [feat, dst]
            h1 = big.tile([P, T * P], f32)     # post-BN/relu layer-1 output
            h16_shard = dram.tile([SH, P], f16)
            h16_full = dram.tile([NN, P], f16, addr_space="Shared")

            gq = [0]

            def gconv_layer(table_full, W_sb, s1_cols, s2_cols):
                """SpMM + W matmul; fills hpre and the per-tile stat columns."""
                srcA = table_full[0:SPLIT, :]
                srcB = table_full[SPLIT:NN, :]
                for ip, (t0, t1) in enumerate(pairs):
                    tl = [t0] if t0 == t1 else [t0, t1]
                    bufs = {}
                    for h, (Cc, offs, sv) in enumerate(
                        ((CA, meta["offs_a"], srcA), (CB, meta["offs_b"], srcB))
                    ):
                        col, wcols, nidx = offs[ip]
                        g = gat.tile([P, 2 * Cc, P], f16, tag=f"g{h}")
                        nch = nidx // P
                        step = max(1, GATHER_MAX_IDX // P)
                        for c0 in range(0, nch, step):
                            c1 = min(c0 + step, nch)
                            nc.gpsimd.dma_gather(
                                g[:, c0:c1, :],
                                sv,
                                idx_sb[:, col + c0 * 8 : col + c1 * 8],
                                (c1 - c0) * P,
                                (c1 - c0) * P,
                                P,
                                single_packet=GATHER_SINGLE_PACKET,
                                queue_num=gq[0] % N_SWDGE_QUEUES,
                            )
                            gq[0] += 1
                        bufs[h] = g
                    for ti, t in enumerate(tl):
                        mT = ps.tile([P, P], f32, tag="mT")
                        cid0 = t * (CA + CB)
                        NCHT = CA + CB
                        # one-hot selectors for the tile's chunks in one DVE op
                        sel = wrk.tile([P, NCHT, P], f16, tag="sel", bufs=3)
                        nc.vector.tensor_tensor(
                            out=sel[:],
                            in0=slot_sb[:, cid0 : cid0 + NCHT][:, :, None]
                            .to_broadcast([P, NCHT, P]),
                            in1=iota[:, None, :].to_broadcast([P, NCHT, P]),
                            op=Alu.is_equal,
                        )
                        for c in range(NCHT):
                            h, cc = (0, c) if c < CA else (1, c - CA)
                            Cc = CA if h == 0 else CB
                            nc.tensor.matmul(
                                out=mT[:],
                                lhsT=bufs[h][:, ti * Cc + cc, :],
                                rhs=sel[:, c, :],
                                start=(c == 0),
                                stop=(c == NCHT - 1),
                            )
                        # evacuate with d_in column scaling
                        mTs = wrk.tile([P, P], f32, tag="mTs")
                        nc.vector.tensor_tensor(
                            out=mTs[:],
                            in0=mT[:],
                            in1=din_bc[:, t * P : (t + 1) * P],
                            op=Alu.mult,
                        )
                        hp = ps.tile([P, P], f32, tag="hp")
                        nc.tensor.matmul(
                            out=hp[:], lhsT=W_sb[:], rhs=mTs[:], start=True, stop=True
                        )
                        # evacuate + per-feature partial sums for BN
                        nc.vector.tensor_scalar(
                            hpre[:, t * P : (t + 1) * P],
                            hp[:],
                            1.0,
                            None,
                            Alu.mult,
                            Alu.add,
                            accum_out=s1_cols[:, t : t + 1],
                        )
                        sq = wrk.tile([P, P], f16, tag="sq")
                        nc.scalar.activation(
                            sq[:],
                            hpre[:, t * P : (t + 1) * P],
                            Act.Square,
                            accum_out=s2_cols[:, t : t + 1],
                        )

            def bn_coeffs(s1_cols, s2_cols, gam, bet, tag):
                """AllReduce partial sums -> per-feature scale a, shift c."""
                stats_in = dram.tile([P, 2], f32, name=f"stats_in_{tag}")
                stats_out = dram.tile(
                    [P, 2], f32, addr_space="Shared", name=f"stats_out_{tag}"
                )
                pack = wrk.tile([P, 2], f32, tag="pack")
                nc.vector.tensor_reduce(
                    pack[:, 0:1], s1_cols[:], axis=mybir.AxisListType.X, op=Alu.add
                )
                nc.vector.tensor_reduce(
                    pack[:, 1:2], s2_cols[:], axis=mybir.AxisListType.X, op=Alu.add
                )
                nc.sync.dma_start(stats_in[:], pack[:])
                nc.gpsimd.collective_compute(
                    "AllReduce",
                    Alu.add,
                    replica_groups=[list(range(N_CORES))],
                    ins=[stats_in.opt()],
                    outs=[stats_out.opt()],
                )
                glob = wrk.tile([P, 2], f32, tag="glob")
                nc.sync.dma_start(glob[:], stats_out[:])
                mo = wrk.tile([P, 4], f32, tag="mo")
                # mo: 0=mu 1=E[h^2] 2=var+eps 3=scratch
                nc.vector.tensor_scalar(mo[:, 0:2], glob[:], 1.0 / NN, None, Alu.mult)
                nc.vector.tensor_tensor(
                    out=mo[:, 3:4], in0=mo[:, 0:1], in1=mo[:, 0:1], op=Alu.mult
                )
                nc.vector.tensor_tensor(
                    out=mo[:, 2:3], in0=mo[:, 1:2], in1=mo[:, 3:4], op=Alu.subtract
                )
                nc.vector.tensor_scalar_add(mo[:, 2:3], mo[:, 2:3], EPS)
                nc.scalar.sqrt(mo[:, 2:3], mo[:, 2:3])
                a_c = cst.tile([P, 2], f32, name=f"a_c_{gam.name}")
                nc.vector.reciprocal(a_c[:, 0:1], mo[:, 2:3])
                nc.vector.tensor_tensor(
                    out=a_c[:, 0:1], in0=a_c[:, 0:1], in1=gam[:], op=Alu.mult
                )
                nc.vector.tensor_tensor(
                    out=a_c[:, 1:2], in0=a_c[:, 0:1], in1=mo[:, 0:1], op=Alu.mult
                )
                nc.vector.tensor_tensor(
                    out=a_c[:, 1:2], in0=bet[:], in1=a_c[:, 1:2], op=Alu.subtract
                )
                return a_c

            # ================= layer 1 =================
            s1a = cst.tile([P, T], f32)
            s2a = cst.tile([P, T], f32)
            gconv_layer(x16_full, W1s, s1a, s2a)
            ac1 = bn_coeffs(s1a, s2a, gm1s, bt1s, "l1")

            # BN + relu -> h1; build fp16 normalized layer-2 table
            for t in range(T):
                r = rows_of(t)
                nc.scalar.activation(
                    h1[:, t * P : (t + 1) * P],
                    hpre[:, t * P : (t + 1) * P],
                    Act.Relu,
                    bias=ac1[:, 1:2],
                    scale=ac1[:, 0:1],
                )
                tp = ps.tile([P, P], f32, tag="tp")
                nc.tensor.transpose(
                    out=tp[:], in_=h1[:, t * P : (t + 1) * P], identity=ident[:]
                )
                st = wrk.tile([P, P], f16, tag="xstage")
                nc.vector.tensor_scalar(
                    st[:r, :], tp[:r, :], d_out[:r, t : t + 1], None, Alu.mult
                )
                nc.sync.dma_start(h16_shard[t * P : t * P + r, :], st[:r, :])
            nc.gpsimd.collective_compute(
                "AllGather",
                Alu.bypass,
                replica_groups=[list(range(N_CORES))],
                ins=[h16_shard.opt()],
                outs=[h16_full.opt()],
            )

            # ================= layer 2 =================
            s1b = cst.tile([P, T], f32)
            s2b = cst.tile([P, T], f32)
            gconv_layer(h16_full, W2s, s1b, s2b)
            ac2 = bn_coeffs(s1b, s2b, gm2s, bt2s, "l2")

            for t in range(T):
                r = rows_of(t)
                h2 = wrk.tile([P, P], f32, tag="h2")
                nc.scalar.activation(
                    h2[:],
                    hpre[:, t * P : (t + 1) * P],
                    Act.Identity,
                    bias=ac2[:, 1:2],
                    scale=ac2[:, 0:1],
                )
                nc.vector.tensor_tensor(
                    out=h2[:], in0=h2[:], in1=h1[:, t * P : (t + 1) * P], op=Alu.add
                )
                nc.scalar.activation(h2[:], h2[:], Act.Relu)
                tp = ps.tile([P, P], f32, tag="tp")
                nc.tensor.transpose(out=tp[:], in_=h2[:], identity=ident[:])
                ot = wrk.tile([P, P], f32, tag="ostage")
                nc.vector.tensor_copy(ot[:r, :], tp[:r, :])
                nc.sync.dma_start(out_t[t * P : t * P + r, :], ot[:r, :])

    nc.compile()
    return nc


# ---------------------------------------------------------------------------


_CACHE = {}


def _get_program(meta):
    key = (meta["SH"], meta["T"], meta["CA"], meta["CB"], meta["idx_cols"])
    if key not in _CACHE:
        _CACHE[key] = _build_program(meta)
    return _CACHE[key]


def kernel(**inputs):
    x = np.asarray(inputs["x"], np.float32)
    src = np.asarray(inputs["src"])
    dst = np.asarray(inputs["dst"])
    n_nodes = x.shape[0]

    meta, cores = _host_prep(src, dst, n_nodes)
    nc = _get_program(meta)

    SH = meta["SH"]
    iota = np.tile(np.arange(P, dtype=np.float16), (P, 1))
    in_maps = []
    for k in range(N_CORES):
        c = cores[k]
        in_maps.append(
            {
                "x_shard": np.ascontiguousarray(x[k * SH : (k + 1) * SH]),
                "W1": np.asarray(inputs["W1"], np.float32),
                "W2": np.asarray(inputs["W2"], np.float32),
                "gamma1": np.asarray(inputs["gamma1"], np.float32).reshape(P, 1),
                "beta1": np.asarray(inputs["beta1"], np.float32).reshape(P, 1),
                "gamma2": np.asarray(inputs["gamma2"], np.float32).reshape(P, 1),
                "beta2": np.asarray(inputs["beta2"], np.float32).reshape(P, 1),
                "iota": iota,
                "idx_img": c["idx_img"],
                "slotT": c["slotT"],
                "deg_out": c["deg_out"],
                "deg_in": c["deg_in"],
            }
        )

    from concourse.bass_utils import run_bass_kernel_spmd

    res = run_bass_kernel_spmd(nc, in_maps, core_ids=list(range(N_CORES)))
    out = np.concatenate([res.results[k]["out"] for k in range(N_CORES)], axis=0)
    return out.astype(np.float32)


# revision 29
# speedup vs baseline: 2.2840x; 1.0406x over previous
"""GCN encoder (2-layer, BN, residual) on 8 Trainium2 NeuronCores.

Sharding: nodes partitioned contiguously across 8 cores (6250 each). Edges
bucketed by dst shard on host (integer-only preprocessing: bucket/sort/pad
edge indices, degree counts via bincount). All float math runs on device:

  - per-node norm d_out=rsqrt(clip(outdeg,1)) folded into an fp16 copy of the
    gather table (x*d_out, AllGathered to every core)
  - SpMM: dma_gather of 128-edge chunks (rows->partitions) + one-hot selector
    matmul on PE accumulating m^T[feat, dst] in PSUM; selector built on DVE
    from iota==slot compare (exact 0/1 entries)
  - d_in applied via a broadcast matrix during PSUM evacuation
  - W matmul with W as the stationary operand keeps the [feat, dst] layout so
    BN (per-feature affine) uses per-partition ACT scale/bias + fused ReLU
  - BN stats: per-core partial sums + 1KB AllReduce
  - layer-2 table: h1*d_out cast fp16, AllGathered
"""

import sys

sys.path.insert(0, "/opt/trn_rl_repo")

import numpy as np

P = 128
N_CORES = 8
EPS = 1e-5

# compute dtype for gather tables / selectors / segment matmul
_F16 = "float16"

# dma_gather tuning (device crashes observed for very large single calls)
GATHER_SINGLE_PACKET = False
GATHER_MAX_IDX = 768  # max indices per dma_gather instruction (larger crashes device)
DMA_SCRATCH = 32768  # per-partition SWDGE descriptor-ring carveout
N_SWDGE_QUEUES = 4  # each queue runs on its own Q7 core pair -> parallel desc-gen


def _cdiv(a, b):
    return -(-a // b)


# ---------------------------------------------------------------------------
# host-side integer preprocessing (indices only; no float arithmetic on data)
# ---------------------------------------------------------------------------


def _wrap_idx_image(idx_list):
    """int16 index list (len % 16 == 0) -> [128, len/16] SBUF image.

    dma_gather reads idx i from partition i%16, free slot i//16; the 16-row
    pattern must be replicated 8x across the 128 partitions (one per Q7 core).
    """
    n = idx_list.shape[0]
    assert n % 16 == 0
    img16 = idx_list.reshape(n // 16, 16).T  # [16, n/16]
    return np.tile(img16, (8, 1)).astype(np.int16)  # [128, n/16]


def _host_prep(src, dst, n_nodes):
    """Bucket edges by (dst shard, dst tile, src half); pad to uniform chunk
    capacities so all 8 cores run one identical program."""
    NC = N_CORES
    SH = n_nodes // NC
    assert SH * NC == n_nodes
    T = _cdiv(SH, P)
    SPLIT = n_nodes // 2
    assert SPLIT < 32768 and (n_nodes - SPLIT) <= 32768

    src = np.asarray(src, np.int64)
    dst = np.asarray(dst, np.int64)

    per_core = []
    CA = CB = 1
    for k in range(NC):
        m = (dst >= k * SH) & (dst < (k + 1) * SH)
        s = src[m]
        dl = dst[m] - k * SH
        t_idx = dl // P
        slot = dl % P
        half = (s >= SPLIT).astype(np.int64)
        idxval = np.where(half == 1, s - SPLIT, s)
        per_core.append((t_idx, half, idxval, slot))
        for t in range(T):
            tm = t_idx == t
            na = int(np.count_nonzero(tm & (half == 0)))
            nb = int(np.count_nonzero(tm & (half == 1)))
            CA = max(CA, _cdiv(na, P))
            CB = max(CB, _cdiv(nb, P))

    n_chunks = T * (CA + CB)
    pairs = [(2 * b, min(2 * b + 1, T - 1)) for b in range(_cdiv(T, 2))]

    cores = []
    for k in range(NC):
        t_idx, half, idxval, slot = per_core[k]
        A_idx = np.zeros((T, CA * P), np.int16)
        B_idx = np.zeros((T, CB * P), np.int16)
        # pad slot = 255: one-hot (iota==slot) never fires -> zero column
        slots = np.full((n_chunks, P), 255.0, np.float16)
        for t in range(T):
            tm = t_idx == t
            for h, (Cc, buf) in enumerate(((CA, A_idx), (CB, B_idx))):
                hm = tm & (half == h)
                iv = idxval[hm]
                sl = slot[hm]
                n = iv.shape[0]
                buf[t, :n] = iv.astype(np.int16)
                base = t * (CA + CB) + (0 if h == 0 else CA)
                for c in range(Cc):
                    lo, hi = c * P, min((c + 1) * P, n)
                    if hi > lo:
                        slots[base + c, : hi - lo] = sl[lo:hi].astype(np.float16)

        # gather-call index images: one A call + one B call per tile pair
        imgs = []
        offs_a, offs_b = [], []
        col = 0
        for t0, t1 in pairs:
            tl = [t0] if t0 == t1 else [t0, t1]
            for h, (Cc, buf, offs) in enumerate(
                ((CA, A_idx, offs_a), (CB, B_idx, offs_b))
            ):
                lst = np.concatenate([buf[t] for t in tl])
                img = _wrap_idx_image(lst)
                offs.append((col, img.shape[1], len(tl) * Cc * P))
                col += img.shape[1]
                imgs.append(img)
        idx_img = np.concatenate(imgs, axis=1)  # [128, col]

        # degree counts (integers), tile-column layout [P, T], pad rows deg=1
        outdeg = np.bincount(src, minlength=n_nodes).astype(np.int64)
        indeg = np.bincount(dst, minlength=n_nodes).astype(np.int64)
        mine = slice(k * SH, (k + 1) * SH)

        def _cols(d):
            v = np.ones(T * P, np.float32)
            v[:SH] = d[mine].astype(np.float32)
            return v.reshape(T, P).T.copy()  # [P, T]

        cores.append(
            dict(
                idx_img=idx_img,
                slotT=slots.T.copy(),  # [P, n_chunks] fp16
                deg_out=_cols(outdeg),
                deg_in=_cols(indeg),
                offs_a=offs_a,
                offs_b=offs_b,
            )
        )

    meta = dict(
        SH=SH,
        T=T,
        SPLIT=SPLIT,
        CA=CA,
        CB=CB,
        n_chunks=n_chunks,
        pairs=pairs,
        idx_cols=cores[0]["idx_img"].shape[1],
        n_nodes=n_nodes,
        # call offsets are identical across cores by construction
        offs_a=cores[0]["offs_a"],
        offs_b=cores[0]["offs_b"],
    )
    for c in cores[1:]:
        assert c["offs_a"] == meta["offs_a"] and c["offs_b"] == meta["offs_b"]
        assert c["idx_img"].shape == cores[0]["idx_img"].shape
    return meta, cores


# ---------------------------------------------------------------------------
# device program (identical on all cores; all data-dependence through SBUF)
# ---------------------------------------------------------------------------


def _build_program(meta):
    import concourse.bacc as bacc
    import concourse.bass as bass
    import concourse.tile as tile
    from concourse import mybir
    from concourse.masks import make_identity

    f32 = mybir.dt.float32
    f16 = getattr(mybir.dt, _F16)
    Alu = mybir.AluOpType
    Act = mybir.ActivationFunctionType

    SH, T, SPLIT = meta["SH"], meta["T"], meta["SPLIT"]
    CA, CB = meta["CA"], meta["CB"]
    NCH = meta["n_chunks"]
    NN = meta["n_nodes"]
    pairs = meta["pairs"]
    rows_of = lambda t: min(P, SH - t * P)

    nc = bacc.Bacc(
        "TRN2",
        target_bir_lowering=False,
        debug=False,
        num_devices=N_CORES,
        dynamic_dma_scratch_size=DMA_SCRATCH,
        num_swdge_queues=N_SWDGE_QUEUES,
    )

    # ---- I/O -------------------------------------------------------------
    SHP = T * P  # shard rows padded to a tile multiple
    x_shard = nc.dram_tensor("x_shard", [SHP, P], f32, kind="ExternalInput")
    W1_t = nc.dram_tensor("W1", [P, P], f32, kind="ExternalInput")
    W2_t = nc.dram_tensor("W2", [P, P], f32, kind="ExternalInput")
    gm1 = nc.dram_tensor("gamma1", [P, 1], f32, kind="ExternalInput")
    bt1 = nc.dram_tensor("beta1", [P, 1], f32, kind="ExternalInput")
    gm2 = nc.dram_tensor("gamma2", [P, 1], f32, kind="ExternalInput")
    bt2 = nc.dram_tensor("beta2", [P, 1], f32, kind="ExternalInput")
    iota_t = nc.dram_tensor("iota", [P, P], f16, kind="ExternalInput")
    idx_t = nc.dram_tensor("idx_img", [P, meta["idx_cols"]], mybir.dt.int16,
                           kind="ExternalInput")
    slot_t = nc.dram_tensor("slotT", [P, NCH], f16, kind="ExternalInput")
    dego_t = nc.dram_tensor("deg_out", [P, T], f32, kind="ExternalInput")
    degi_t = nc.dram_tensor("deg_in", [P, T], f32, kind="ExternalInput")
    out_t = nc.dram_tensor("out", [SHP, P], f32, kind="ExternalOutput")

    with tile.TileContext(nc) as tc:
        with (
            tc.tile_pool(name="cst", bufs=1) as cst,
            tc.tile_pool(name="big", bufs=1) as big,
            tc.tile_pool(name="gat", bufs=2) as gat,
            tc.tile_pool(name="wrk", bufs=3) as wrk,
            tc.tile_pool(name="ps", bufs=2, space="PSUM") as ps,
            tc.tile_pool(name="dram", bufs=1, space="DRAM") as dram,
        ):
            # ---- degree normalizers first: the x16 table cast + AllGather
            # is the serial head of the kernel, so issue it before the bulk
            # static-data loads.
            d_out = cst.tile([P, T], f32)
            d_in = cst.tile([P, T], f32)
            for deg_dram, d_sb in ((dego_t, d_out), (degi_t, d_in)):
                raw = wrk.tile([P, T], f32, tag="degraw")
                nc.sync.dma_start(raw[:], deg_dram[:])
                nc.vector.tensor_scalar_max(raw[:], raw[:], 1.0)
                nc.scalar.sqrt(raw[:], raw[:])
                nc.vector.reciprocal(d_sb[:], raw[:])

            # ---- fp16 normalized gather table for layer 1 ---------------
            # bulk load -> one batched scale+cast -> bulk store -> AllGather
            x16_shard = dram.tile([SHP, P], f16)
            x16_full = dram.tile([NN, P], f16, addr_space="Shared")
            xall = big.tile([P, T, P], f32, tag="bigf32")
            nc.sync.dma_start(xall[:], x_shard.rearrange("(t p) f -> p t f", p=P))
            x16all = big.tile([P, T, P], f16, tag="big16a")
            nc.vector.tensor_tensor(
                out=x16all[:],
                in0=xall[:],
                in1=d_out[:, :, None].to_broadcast([P, T, P]),
                op=Alu.mult,
            )
            nc.sync.dma_start(
                x16_shard.rearrange("(t p) f -> p t f", p=P), x16all[:]
            )
            nc.gpsimd.collective_compute(
                "AllGather",
                Alu.bypass,
                replica_groups=[list(range(N_CORES))],
                ins=[x16_shard[0:SH, :].opt()],
                outs=[x16_full.opt()],
            )

            # ---- constants / static data --------------------------------
            ident = cst.tile([P, P], f32)
            make_identity(nc, ident[:])
            W1s = cst.tile([P, P], f32)
            W2s = cst.tile([P, P], f32)
            iota = cst.tile([P, P], f16)
            nc.sync.dma_start(W1s[:], W1_t[:])
            nc.sync.dma_start(W2s[:], W2_t[:])
            nc.sync.dma_start(iota[:], iota_t[:])
            idx_sb = cst.tile([P, meta["idx_cols"]], mybir.dt.int16)
            nc.sync.dma_start(idx_sb[:], idx_t[:])
            slot_sb = cst.tile([P, NCH], f16)
            nc.sync.dma_start(slot_sb[:], slot_t[:])
            gm1s = cst.tile([P, 1], f32)
            bt1s = cst.tile([P, 1], f32)
            gm2s = cst.tile([P, 1], f32)
            bt2s = cst.tile([P, 1], f32)
            nc.sync.dma_start(gm1s[:], gm1[:])
            nc.sync.dma_start(bt1s[:], bt1[:])
            nc.sync.dma_start(gm2s[:], gm2[:])
            nc.sync.dma_start(bt2s[:], bt2[:])
            # fp16 copies of the weight matrices (mT is fp16-derived anyway)
            W1h = cst.tile([P, P], f16)
            W2h = cst.tile([P, P], f16)
            nc.vector.tensor_copy(W1h[:], W1s[:])
            nc.vector.tensor_copy(W2h[:], W2s[:])
            ident16 = cst.tile([P, P], f16)
            nc.vector.tensor_copy(ident16[:], ident[:])

            # d_in broadcast rows: din_bc[:, t*P+j] = d_in[j, t] for all rows
            din_bc = big.tile([P, T * P], f32)
            for t in range(T):
                bc_ps = ps.tile([P, P], f32, tag="tp")
                nc.tensor.transpose(
                    out=bc_ps[:],
                    in_=d_in[:, t : t + 1].to_broadcast([P, P]),
                    identity=ident[:],
                )
                nc.vector.tensor_copy(din_bc[:, t * P : (t + 1) * P], bc_ps[:])

            # persistent stores
            hpre = big.tile([P, T * P], f32)   # pre-BN activations [feat, dst]
            h1 = big.tile([P, T * P], f32)     # post-BN/relu layer-1 output
            h16_shard = dram.tile([SHP, P], f16)
            h16_full = dram.tile([NN, P], f16, addr_space="Shared")

            gq = [0]

            def gconv_layer(table_full, W_sb, s1_cols, s2_cols):
                """SpMM + W matmul; fills hpre and the per-tile stat columns."""
                srcA = table_full[0:SPLIT, :]
                srcB = table_full[SPLIT:NN, :]
                for ip, (t0, t1) in enumerate(pairs):
                    tl = [t0] if t0 == t1 else [t0, t1]
                    bufs = {}
                    for h, (Cc, offs, sv) in enumerate(
                        ((CA, meta["offs_a"], srcA), (CB, meta["offs_b"], srcB))
                    ):
                        col, wcols, nidx = offs[ip]
                        g = gat.tile([P, 2 * Cc, P], f16, tag=f"g{h}")
                        nch = nidx // P
                        step = max(1, GATHER_MAX_IDX // P)
                        for c0 in range(0, nch, step):
                            c1 = min(c0 + step, nch)
                            nc.gpsimd.dma_gather(
                                g[:, c0:c1, :],
                                sv,
                                idx_sb[:, col + c0 * 8 : col + c1 * 8],
                                (c1 - c0) * P,
                                (c1 - c0) * P,
                                P,
                                single_packet=GATHER_SINGLE_PACKET,
                                queue_num=gq[0] % N_SWDGE_QUEUES,
                            )
                            gq[0] += 1
                        bufs[h] = g
                    NCHT = CA + CB
                    mTs_ps = []
                    sels = []
                    for ti, t in enumerate(tl):
                        cid0 = t * NCHT
                        sel = wrk.tile([P, NCHT, P], f16, tag="sel", bufs=4,
                                       name=f"sel{ti}")
                        nc.vector.tensor_tensor(
                            out=sel[:],
                            in0=slot_sb[:, cid0 : cid0 + NCHT][:, :, None]
                            .to_broadcast([P, NCHT, P]),
                            in1=iota[:, None, :].to_broadcast([P, NCHT, P]),
                            op=Alu.is_equal,
                        )
                        sels.append(sel)
                        mTs_ps.append(ps.tile([P, P], f32, tag="mT", bufs=4,
                                              name=f"mT{ti}"))
                    # interleave the two tiles' accumulation chains so the PE
                    # alternates PSUM banks (hides write-commit latency)
                    for c in range(NCHT):
                        h, cc = (0, c) if c < CA else (1, c - CA)
                        Cc = CA if h == 0 else CB
                        for ti in range(len(tl)):
                            nc.tensor.matmul(
                                out=mTs_ps[ti][:],
                                lhsT=bufs[h][:, ti * Cc + cc, :],
                                rhs=sels[ti][:, c, :],
                                start=(c == 0),
                                stop=(c == NCHT - 1),
                            )
                    for ti, t in enumerate(tl):
                        # evacuate with d_in column scaling (fp16 for the W mm)
                        mTs = wrk.tile([P, P], f16, tag="mTs")
                        nc.vector.tensor_tensor(
                            out=mTs[:],
                            in0=mTs_ps[ti][:],
                            in1=din_bc[:, t * P : (t + 1) * P],
                            op=Alu.mult,
                        )
                        hp = ps.tile([P, P], f32, tag="hp")
                        nc.tensor.matmul(
                            out=hp[:], lhsT=W_sb[:], rhs=mTs[:], start=True, stop=True
                        )
                        # evacuate + per-feature partial sums for BN
                        nc.vector.tensor_scalar(
                            hpre[:, t * P : (t + 1) * P],
                            hp[:],
                            1.0,
                            None,
                            Alu.mult,
                            Alu.add,
                            accum_out=s1_cols[:, t : t + 1],
                        )
                        sq = wrk.tile([P, P], f16, tag="sq")
                        nc.scalar.activation(
                            sq[:],
                            hpre[:, t * P : (t + 1) * P],
                            Act.Square,
                            accum_out=s2_cols[:, t : t + 1],
                        )

            def bn_coeffs(s1_cols, s2_cols, gam, bet, tag):
                """AllReduce partial sums -> per-feature scale a, shift c."""
                stats_in = dram.tile([P, 2], f32, name=f"stats_in_{tag}")
                stats_out = dram.tile(
                    [P, 2], f32, addr_space="Shared", name=f"stats_out_{tag}"
                )
                pack = wrk.tile([P, 2], f32, tag="pack")
                nc.vector.tensor_reduce(
                    pack[:, 0:1], s1_cols[:], axis=mybir.AxisListType.X, op=Alu.add
                )
                nc.vector.tensor_reduce(
                    pack[:, 1:2], s2_cols[:], axis=mybir.AxisListType.X, op=Alu.add
                )
                nc.sync.dma_start(stats_in[:], pack[:])
                nc.gpsimd.collective_compute(
                    "AllReduce",
                    Alu.add,
                    replica_groups=[list(range(N_CORES))],
                    ins=[stats_in.opt()],
                    outs=[stats_out.opt()],
                )
                glob = wrk.tile([P, 2], f32, tag="glob")
                nc.sync.dma_start(glob[:], stats_out[:])
                mo = wrk.tile([P, 4], f32, tag="mo")
                # mo: 0=mu 1=E[h^2] 2=var+eps 3=scratch
                nc.vector.tensor_scalar(mo[:, 0:2], glob[:], 1.0 / NN, None, Alu.mult)
                nc.vector.tensor_tensor(
                    out=mo[:, 3:4], in0=mo[:, 0:1], in1=mo[:, 0:1], op=Alu.mult
                )
                nc.vector.tensor_tensor(
                    out=mo[:, 2:3], in0=mo[:, 1:2], in1=mo[:, 3:4], op=Alu.subtract
                )
                nc.vector.tensor_scalar_add(mo[:, 2:3], mo[:, 2:3], EPS)
                nc.scalar.sqrt(mo[:, 2:3], mo[:, 2:3])
                a_c = cst.tile([P, 2], f32, name=f"a_c_{gam.name}")
                nc.vector.reciprocal(a_c[:, 0:1], mo[:, 2:3])
                nc.vector.tensor_tensor(
                    out=a_c[:, 0:1], in0=a_c[:, 0:1], in1=gam[:], op=Alu.mult
                )
                nc.vector.tensor_tensor(
                    out=a_c[:, 1:2], in0=a_c[:, 0:1], in1=mo[:, 0:1], op=Alu.mult
                )
                nc.vector.tensor_tensor(
                    out=a_c[:, 1:2], in0=bet[:], in1=a_c[:, 1:2], op=Alu.subtract
                )
                return a_c

            # ================= layer 1 =================
            s1a = cst.tile([P, T], f32)
            s2a = cst.tile([P, T], f32)
            gconv_layer(x16_full, W1h, s1a, s2a)
            ac1 = bn_coeffs(s1a, s2a, gm1s, bt1s, "l1")

            # BN + relu -> h1 in one batched ACT op, then fp16 transposes
            # and one bulk store of the normalized layer-2 table
            nc.scalar.activation(
                h1[:], hpre[:], Act.Relu, bias=ac1[:, 1:2], scale=ac1[:, 0:1]
            )
            h1_16 = big.tile([P, T * P], f16, tag="big16a")
            nc.vector.tensor_copy(h1_16[:], h1[:])
            st_all = big.tile([P, T, P], f16, tag="big16b")
            for t in range(T):
                tp = ps.tile([P, P], f16, tag="tp")
                nc.tensor.transpose(
                    out=tp[:],
                    in_=h1_16[:, t * P : (t + 1) * P],
                    identity=ident16[:],
                )
                nc.vector.tensor_scalar(
                    st_all[:, t, :], tp[:], d_out[:, t : t + 1], None, Alu.mult
                )
            nc.sync.dma_start(
                h16_shard.rearrange("(t p) f -> p t f", p=P), st_all[:]
            )
            nc.gpsimd.collective_compute(
                "AllGather",
                Alu.bypass,
                replica_groups=[list(range(N_CORES))],
                ins=[h16_shard[0:SH, :].opt()],
                outs=[h16_full.opt()],
            )

            # ================= layer 2 =================
            s1b = cst.tile([P, T], f32)
            s2b = cst.tile([P, T], f32)
            gconv_layer(h16_full, W2s, s1b, s2b)
            ac2 = bn_coeffs(s1b, s2b, gm2s, bt2s, "l2")

            # batched: h2 = ac2*hpre + c2 (in place), r16 = fp16(relu(h2+h1)),
            # fp16 transposes, then one bulk store of the output rows
            nc.scalar.activation(
                hpre[:], hpre[:], Act.Identity, bias=ac2[:, 1:2], scale=ac2[:, 0:1]
            )
            nc.vector.tensor_tensor(out=hpre[:], in0=hpre[:], in1=h1[:], op=Alu.add)
            r16 = big.tile([P, T * P], f16, tag="big16a")
            nc.vector.tensor_scalar(r16[:], hpre[:], 0.0, None, Alu.max)
            oall = big.tile([P, T, P], f32, tag="bigf32")
            for t in range(T):
                tp = ps.tile([P, P], f16, tag="tp")
                nc.tensor.transpose(
                    out=tp[:],
                    in_=r16[:, t * P : (t + 1) * P],
                    identity=ident16[:],
                )
                nc.vector.tensor_copy(oall[:, t, :], tp[:])
            nc.sync.dma_start(out_t.rearrange("(t p) f -> p t f", p=P), oall[:])

    nc.compile()
    return nc


# ---------------------------------------------------------------------------


_CACHE = {}


def _get_program(meta):
    key = (meta["SH"], meta["T"], meta["CA"], meta["CB"], meta["idx_cols"])
    if key not in _CACHE:
        _CACHE[key] = _build_program(meta)
    return _CACHE[key]


def _build_in_maps(meta, cores, inputs):
    x = np.asarray(inputs["x"], np.float32)
    SH, T = meta["SH"], meta["T"]
    SHP = T * P
    iota = np.tile(np.arange(P, dtype=np.float16), (P, 1))
    in_maps = []
    for k in range(N_CORES):
        c = cores[k]
        xs = np.zeros((SHP, P), np.float32)
        xs[:SH] = x[k * SH : (k + 1) * SH]
        in_maps.append(
            {
                "x_shard": xs,
                "W1": np.asarray(inputs["W1"], np.float32),
                "W2": np.asarray(inputs["W2"], np.float32),
                "gamma1": np.asarray(inputs["gamma1"], np.float32).reshape(P, 1),
                "beta1": np.asarray(inputs["beta1"], np.float32).reshape(P, 1),
                "gamma2": np.asarray(inputs["gamma2"], np.float32).reshape(P, 1),
                "beta2": np.asarray(inputs["beta2"], np.float32).reshape(P, 1),
                "iota": iota,
                "idx_img": c["idx_img"],
                "slotT": c["slotT"],
                "deg_out": c["deg_out"],
                "deg_in": c["deg_in"],
            }
        )
    return in_maps


def kernel(**inputs):
    x = np.asarray(inputs["x"], np.float32)
    src = np.asarray(inputs["src"])
    dst = np.asarray(inputs["dst"])
    n_nodes = x.shape[0]

    meta, cores = _host_prep(src, dst, n_nodes)
    nc = _get_program(meta)
    in_maps = _build_in_maps(meta, cores, inputs)

    from concourse.bass_utils import run_bass_kernel_spmd

    res = run_bass_kernel_spmd(nc, in_maps, core_ids=list(range(N_CORES)))
    SH = meta["SH"]
    out = np.concatenate(
        [res.results[k]["out"][:SH] for k in range(N_CORES)], axis=0
    )
    return out.astype(np.float32)


# revision 30
# speedup vs baseline: 2.3183x; 1.0150x over previous
"""GCN encoder (2-layer, BN, residual) on 8 Trainium2 NeuronCores.

Sharding: nodes partitioned contiguously across 8 cores (6250 each). Edges
bucketed by dst shard on host (integer-only preprocessing: bucket/sort/pad
edge indices, degree counts via bincount). All float math runs on device:

  - per-node norm d_out=rsqrt(clip(outdeg,1)) folded into an fp16 copy of the
    gather table (x*d_out, AllGathered to every core)
  - SpMM: dma_gather of 128-edge chunks (rows->partitions) + one-hot selector
    matmul on PE accumulating m^T[feat, dst] in PSUM; selector built on DVE
    from iota==slot compare (exact 0/1 entries)
  - d_in applied via a broadcast matrix during PSUM evacuation
  - W matmul with W as the stationary operand keeps the [feat, dst] layout so
    BN (per-feature affine) uses per-partition ACT scale/bias + fused ReLU
  - BN stats: per-core partial sums + 1KB AllReduce
  - layer-2 table: h1*d_out cast fp16, AllGathered
"""

import sys

sys.path.insert(0, "/opt/trn_rl_repo")

import numpy as np

P = 128
N_CORES = 8
EPS = 1e-5

# compute dtype for gather tables / selectors / segment matmul
_F16 = "bfloat16"

# dma_gather tuning (device crashes observed for very large single calls)
GATHER_SINGLE_PACKET = False
GATHER_MAX_IDX = 768  # max indices per dma_gather instruction (larger crashes device)
DMA_SCRATCH = 32768  # per-partition SWDGE descriptor-ring carveout
N_SWDGE_QUEUES = 4  # each queue runs on its own Q7 core pair -> parallel desc-gen


def _cdiv(a, b):
    return -(-a // b)


# ---------------------------------------------------------------------------
# host-side integer preprocessing (indices only; no float arithmetic on data)
# ---------------------------------------------------------------------------


def _wrap_idx_image(idx_list):
    """int16 index list (len % 16 == 0) -> [128, len/16] SBUF image.

    dma_gather reads idx i from partition i%16, free slot i//16; the 16-row
    pattern must be replicated 8x across the 128 partitions (one per Q7 core).
    """
    n = idx_list.shape[0]
    assert n % 16 == 0
    img16 = idx_list.reshape(n // 16, 16).T  # [16, n/16]
    return np.tile(img16, (8, 1)).astype(np.int16)  # [128, n/16]


def _host_prep(src, dst, n_nodes):
    """Bucket edges by (dst shard, dst tile, src half); pad to uniform chunk
    capacities so all 8 cores run one identical program."""
    NC = N_CORES
    SH = n_nodes // NC
    assert SH * NC == n_nodes
    T = _cdiv(SH, P)
    SPLIT = n_nodes // 2
    assert SPLIT < 32768 and (n_nodes - SPLIT) <= 32768

    src = np.asarray(src, np.int64)
    dst = np.asarray(dst, np.int64)

    per_core = []
    CA = CB = 1
    for k in range(NC):
        m = (dst >= k * SH) & (dst < (k + 1) * SH)
        s = src[m]
        dl = dst[m] - k * SH
        t_idx = dl // P
        slot = dl % P
        half = (s >= SPLIT).astype(np.int64)
        idxval = np.where(half == 1, s - SPLIT, s)
        per_core.append((t_idx, half, idxval, slot))
        for t in range(T):
            tm = t_idx == t
            na = int(np.count_nonzero(tm & (half == 0)))
            nb = int(np.count_nonzero(tm & (half == 1)))
            CA = max(CA, _cdiv(na, P))
            CB = max(CB, _cdiv(nb, P))

    n_chunks = T * (CA + CB)
    pairs = [(2 * b, min(2 * b + 1, T - 1)) for b in range(_cdiv(T, 2))]

    cores = []
    for k in range(NC):
        t_idx, half, idxval, slot = per_core[k]
        A_idx = np.zeros((T, CA * P), np.int16)
        B_idx = np.zeros((T, CB * P), np.int16)
        # pad slot = 255: one-hot (iota==slot) never fires -> zero column
        import ml_dtypes
        slots = np.full((n_chunks, P), 255.0, ml_dtypes.bfloat16)
        for t in range(T):
            tm = t_idx == t
            for h, (Cc, buf) in enumerate(((CA, A_idx), (CB, B_idx))):
                hm = tm & (half == h)
                iv = idxval[hm]
                sl = slot[hm]
                n = iv.shape[0]
                buf[t, :n] = iv.astype(np.int16)
                base = t * (CA + CB) + (0 if h == 0 else CA)
                for c in range(Cc):
                    lo, hi = c * P, min((c + 1) * P, n)
                    if hi > lo:
                        slots[base + c, : hi - lo] = sl[lo:hi].astype(ml_dtypes.bfloat16)

        # gather-call index images: one A call + one B call per tile pair
        imgs = []
        offs_a, offs_b = [], []
        col = 0
        for t0, t1 in pairs:
            tl = [t0] if t0 == t1 else [t0, t1]
            for h, (Cc, buf, offs) in enumerate(
                ((CA, A_idx, offs_a), (CB, B_idx, offs_b))
            ):
                lst = np.concatenate([buf[t] for t in tl])
                img = _wrap_idx_image(lst)
                offs.append((col, img.shape[1], len(tl) * Cc * P))
                col += img.shape[1]
                imgs.append(img)
        idx_img = np.concatenate(imgs, axis=1)  # [128, col]

        # degree counts (integers), tile-column layout [P, T], pad rows deg=1
        outdeg = np.bincount(src, minlength=n_nodes).astype(np.int64)
        indeg = np.bincount(dst, minlength=n_nodes).astype(np.int64)
        mine = slice(k * SH, (k + 1) * SH)

        def _cols(d):
            v = np.ones(T * P, np.float32)
            v[:SH] = d[mine].astype(np.float32)
            return v.reshape(T, P).T.copy()  # [P, T]

        cores.append(
            dict(
                idx_img=idx_img,
                slotT=slots.T.copy(),  # [P, n_chunks] fp16
                deg_out=_cols(outdeg),
                deg_in=_cols(indeg),
                offs_a=offs_a,
                offs_b=offs_b,
            )
        )

    meta = dict(
        SH=SH,
        T=T,
        SPLIT=SPLIT,
        CA=CA,
        CB=CB,
        n_chunks=n_chunks,
        pairs=pairs,
        idx_cols=cores[0]["idx_img"].shape[1],
        n_nodes=n_nodes,
        # call offsets are identical across cores by construction
        offs_a=cores[0]["offs_a"],
        offs_b=cores[0]["offs_b"],
    )
    for c in cores[1:]:
        assert c["offs_a"] == meta["offs_a"] and c["offs_b"] == meta["offs_b"]
        assert c["idx_img"].shape == cores[0]["idx_img"].shape
    return meta, cores


# ---------------------------------------------------------------------------
# device program (identical on all cores; all data-dependence through SBUF)
# ---------------------------------------------------------------------------


def _build_program(meta):
    import concourse.bacc as bacc
    import concourse.bass as bass
    import concourse.tile as tile
    from concourse import mybir
    from concourse.masks import make_identity

    f32 = mybir.dt.float32
    f16 = getattr(mybir.dt, _F16)
    Alu = mybir.AluOpType
    Act = mybir.ActivationFunctionType

    SH, T, SPLIT = meta["SH"], meta["T"], meta["SPLIT"]
    CA, CB = meta["CA"], meta["CB"]
    NCH = meta["n_chunks"]
    NN = meta["n_nodes"]
    pairs = meta["pairs"]
    rows_of = lambda t: min(P, SH - t * P)

    nc = bacc.Bacc(
        "TRN2",
        target_bir_lowering=False,
        debug=False,
        num_devices=N_CORES,
        dynamic_dma_scratch_size=DMA_SCRATCH,
        num_swdge_queues=N_SWDGE_QUEUES,
    )

    # ---- I/O -------------------------------------------------------------
    SHP = T * P  # shard rows padded to a tile multiple
    x_shard = nc.dram_tensor("x_shard", [SHP, P], f32, kind="ExternalInput")
    W1_t = nc.dram_tensor("W1", [P, P], f32, kind="ExternalInput")
    W2_t = nc.dram_tensor("W2", [P, P], f32, kind="ExternalInput")
    gm1 = nc.dram_tensor("gamma1", [P, 1], f32, kind="ExternalInput")
    bt1 = nc.dram_tensor("beta1", [P, 1], f32, kind="ExternalInput")
    gm2 = nc.dram_tensor("gamma2", [P, 1], f32, kind="ExternalInput")
    bt2 = nc.dram_tensor("beta2", [P, 1], f32, kind="ExternalInput")
    iota_t = nc.dram_tensor("iota", [P, P], f16, kind="ExternalInput")
    idx_t = nc.dram_tensor("idx_img", [P, meta["idx_cols"]], mybir.dt.int16,
                           kind="ExternalInput")
    slot_t = nc.dram_tensor("slotT", [P, NCH], f16, kind="ExternalInput")
    dego_t = nc.dram_tensor("deg_out", [P, T], f32, kind="ExternalInput")
    degi_t = nc.dram_tensor("deg_in", [P, T], f32, kind="ExternalInput")
    out_t = nc.dram_tensor("out", [SHP, P], f32, kind="ExternalOutput")

    with tile.TileContext(nc) as tc:
        with (
            tc.tile_pool(name="cst", bufs=1) as cst,
            tc.tile_pool(name="big", bufs=1) as big,
            tc.tile_pool(name="gat", bufs=2) as gat,
            tc.tile_pool(name="wrk", bufs=3) as wrk,
            tc.tile_pool(name="ps", bufs=2, space="PSUM") as ps,
            tc.tile_pool(name="dram", bufs=1, space="DRAM") as dram,
        ):
            # ---- degree normalizers first: the x16 table cast + AllGather
            # is the serial head of the kernel, so issue it before the bulk
            # static-data loads.
            d_out = cst.tile([P, T], f32)
            d_in = cst.tile([P, T], f32)
            for deg_dram, d_sb in ((dego_t, d_out), (degi_t, d_in)):
                raw = wrk.tile([P, T], f32, tag="degraw")
                nc.sync.dma_start(raw[:], deg_dram[:])
                nc.vector.tensor_scalar_max(raw[:], raw[:], 1.0)
                nc.scalar.sqrt(raw[:], raw[:])
                nc.vector.reciprocal(d_sb[:], raw[:])

            # ---- fp16 normalized gather table for layer 1 ---------------
            # bulk load -> one batched scale+cast -> bulk store -> AllGather
            x16_shard = dram.tile([SHP, P], f16)
            x16_full = dram.tile([NN, P], f16, addr_space="Shared")
            xall = big.tile([P, T, P], f32, tag="bigf32")
            nc.sync.dma_start(xall[:], x_shard.rearrange("(t p) f -> p t f", p=P))
            x16all = big.tile([P, T, P], f16, tag="big16a")
            nc.vector.tensor_tensor(
                out=x16all[:],
                in0=xall[:],
                in1=d_out[:, :, None].to_broadcast([P, T, P]),
                op=Alu.mult,
            )
            nc.sync.dma_start(
                x16_shard.rearrange("(t p) f -> p t f", p=P), x16all[:]
            )
            nc.gpsimd.collective_compute(
                "AllGather",
                Alu.bypass,
                replica_groups=[list(range(N_CORES))],
                ins=[x16_shard[0:SH, :].opt()],
                outs=[x16_full.opt()],
            )

            # ---- constants / static data --------------------------------
            ident = cst.tile([P, P], f32)
            make_identity(nc, ident[:])
            W1s = cst.tile([P, P], f32)
            W2s = cst.tile([P, P], f32)
            iota = cst.tile([P, P], f16)
            nc.sync.dma_start(W1s[:], W1_t[:])
            nc.sync.dma_start(W2s[:], W2_t[:])
            nc.sync.dma_start(iota[:], iota_t[:])
            idx_sb = cst.tile([P, meta["idx_cols"]], mybir.dt.int16)
            nc.sync.dma_start(idx_sb[:], idx_t[:])
            slot_sb = cst.tile([P, NCH], f16)
            nc.sync.dma_start(slot_sb[:], slot_t[:])
            gm1s = cst.tile([P, 1], f32)
            bt1s = cst.tile([P, 1], f32)
            gm2s = cst.tile([P, 1], f32)
            bt2s = cst.tile([P, 1], f32)
            nc.sync.dma_start(gm1s[:], gm1[:])
            nc.sync.dma_start(bt1s[:], bt1[:])
            nc.sync.dma_start(gm2s[:], gm2[:])
            nc.sync.dma_start(bt2s[:], bt2[:])
            # fp16 copies of the weight matrices (mT is fp16-derived anyway)
            W1h = cst.tile([P, P], f16)
            W2h = cst.tile([P, P], f16)
            nc.vector.tensor_copy(W1h[:], W1s[:])
            nc.vector.tensor_copy(W2h[:], W2s[:])
            ident16 = cst.tile([P, P], f16)
            nc.vector.tensor_copy(ident16[:], ident[:])

            # d_in broadcast rows: din_bc[:, t*P+j] = d_in[j, t] for all rows
            din_bc = big.tile([P, T * P], f32)
            for t in range(T):
                bc_ps = ps.tile([P, P], f32, tag="tp")
                nc.tensor.transpose(
                    out=bc_ps[:],
                    in_=d_in[:, t : t + 1].to_broadcast([P, P]),
                    identity=ident[:],
                )
                nc.vector.tensor_copy(din_bc[:, t * P : (t + 1) * P], bc_ps[:])

            # persistent stores
            hpre = big.tile([P, T * P], f32)   # pre-BN activations [feat, dst]
            h1 = big.tile([P, T * P], f32)     # post-BN/relu layer-1 output
            h16_shard = dram.tile([SHP, P], f16)
            h16_full = dram.tile([NN, P], f16, addr_space="Shared")

            gq = [0]

            def gconv_layer(table_full, W_sb, s1_cols, s2_cols):
                """SpMM + W matmul; fills hpre and the per-tile stat columns."""
                srcA = table_full[0:SPLIT, :]
                srcB = table_full[SPLIT:NN, :]
                for ip, (t0, t1) in enumerate(pairs):
                    tl = [t0] if t0 == t1 else [t0, t1]
                    bufs = {}
                    for h, (Cc, offs, sv) in enumerate(
                        ((CA, meta["offs_a"], srcA), (CB, meta["offs_b"], srcB))
                    ):
                        col, wcols, nidx = offs[ip]
                        g = gat.tile([P, 2 * Cc, P], f16, tag=f"g{h}")
                        nch = nidx // P
                        step = max(1, GATHER_MAX_IDX // P)
                        for c0 in range(0, nch, step):
                            c1 = min(c0 + step, nch)
                            nc.gpsimd.dma_gather(
                                g[:, c0:c1, :],
                                sv,
                                idx_sb[:, col + c0 * 8 : col + c1 * 8],
                                (c1 - c0) * P,
                                (c1 - c0) * P,
                                P,
                                single_packet=GATHER_SINGLE_PACKET,
                                queue_num=gq[0] % N_SWDGE_QUEUES,
                            )
                            gq[0] += 1
                        bufs[h] = g
                    NCHT = CA + CB
                    mTs_ps = []
                    sels = []
                    for ti, t in enumerate(tl):
                        cid0 = t * NCHT
                        sel = wrk.tile([P, NCHT, P], f16, tag="sel", bufs=4,
                                       name=f"sel{ti}")
                        nc.vector.tensor_tensor(
                            out=sel[:],
                            in0=slot_sb[:, cid0 : cid0 + NCHT][:, :, None]
                            .to_broadcast([P, NCHT, P]),
                            in1=iota[:, None, :].to_broadcast([P, NCHT, P]),
                            op=Alu.is_equal,
                        )
                        sels.append(sel)
                        mTs_ps.append(ps.tile([P, P], f32, tag="mT", bufs=4,
                                              name=f"mT{ti}"))
                    # interleave the two tiles' accumulation chains so the PE
                    # alternates PSUM banks (hides write-commit latency)
                    for c in range(NCHT):
                        h, cc = (0, c) if c < CA else (1, c - CA)
                        Cc = CA if h == 0 else CB
                        for ti in range(len(tl)):
                            nc.tensor.matmul(
                                out=mTs_ps[ti][:],
                                lhsT=bufs[h][:, ti * Cc + cc, :],
                                rhs=sels[ti][:, c, :],
                                start=(c == 0),
                                stop=(c == NCHT - 1),
                            )
                    for ti, t in enumerate(tl):
                        # evacuate with d_in column scaling (fp16 for the W mm)
                        mTs = wrk.tile([P, P], f16, tag="mTs")
                        nc.vector.tensor_tensor(
                            out=mTs[:],
                            in0=mTs_ps[ti][:],
                            in1=din_bc[:, t * P : (t + 1) * P],
                            op=Alu.mult,
                        )
                        hp = ps.tile([P, P], f32, tag="hp")
                        nc.tensor.matmul(
                            out=hp[:], lhsT=W_sb[:], rhs=mTs[:], start=True, stop=True
                        )
                        # evacuate + per-feature partial sums for BN
                        nc.vector.tensor_scalar(
                            hpre[:, t * P : (t + 1) * P],
                            hp[:],
                            1.0,
                            None,
                            Alu.mult,
                            Alu.add,
                            accum_out=s1_cols[:, t : t + 1],
                        )
                        sq = wrk.tile([P, P], f16, tag="sq")
                        nc.scalar.activation(
                            sq[:],
                            hpre[:, t * P : (t + 1) * P],
                            Act.Square,
                            accum_out=s2_cols[:, t : t + 1],
                        )

            def bn_coeffs(s1_cols, s2_cols, gam, bet, tag):
                """AllReduce partial sums -> per-feature scale a, shift c."""
                stats_in = dram.tile([P, 2], f32, name=f"stats_in_{tag}")
                stats_out = dram.tile(
                    [P, 2], f32, addr_space="Shared", name=f"stats_out_{tag}"
                )
                pack = wrk.tile([P, 2], f32, tag="pack")
                nc.vector.tensor_reduce(
                    pack[:, 0:1], s1_cols[:], axis=mybir.AxisListType.X, op=Alu.add
                )
                nc.vector.tensor_reduce(
                    pack[:, 1:2], s2_cols[:], axis=mybir.AxisListType.X, op=Alu.add
                )
                nc.sync.dma_start(stats_in[:], pack[:])
                nc.gpsimd.collective_compute(
                    "AllReduce",
                    Alu.add,
                    replica_groups=[list(range(N_CORES))],
                    ins=[stats_in.opt()],
                    outs=[stats_out.opt()],
                )
                glob = wrk.tile([P, 2], f32, tag="glob")
                nc.sync.dma_start(glob[:], stats_out[:])
                mo = wrk.tile([P, 4], f32, tag="mo")
                # mo: 0=mu 1=E[h^2] 2=var+eps 3=scratch
                nc.vector.tensor_scalar(mo[:, 0:2], glob[:], 1.0 / NN, None, Alu.mult)
                nc.vector.tensor_tensor(
                    out=mo[:, 3:4], in0=mo[:, 0:1], in1=mo[:, 0:1], op=Alu.mult
                )
                nc.vector.tensor_tensor(
                    out=mo[:, 2:3], in0=mo[:, 1:2], in1=mo[:, 3:4], op=Alu.subtract
                )
                nc.vector.tensor_scalar_add(mo[:, 2:3], mo[:, 2:3], EPS)
                nc.scalar.sqrt(mo[:, 2:3], mo[:, 2:3])
                a_c = cst.tile([P, 2], f32, name=f"a_c_{gam.name}")
                nc.vector.reciprocal(a_c[:, 0:1], mo[:, 2:3])
                nc.vector.tensor_tensor(
                    out=a_c[:, 0:1], in0=a_c[:, 0:1], in1=gam[:], op=Alu.mult
                )
                nc.vector.tensor_tensor(
                    out=a_c[:, 1:2], in0=a_c[:, 0:1], in1=mo[:, 0:1], op=Alu.mult
                )
                nc.vector.tensor_tensor(
                    out=a_c[:, 1:2], in0=bet[:], in1=a_c[:, 1:2], op=Alu.subtract
                )
                return a_c

            # ================= layer 1 =================
            s1a = cst.tile([P, T], f32)
            s2a = cst.tile([P, T], f32)
            gconv_layer(x16_full, W1h, s1a, s2a)
            ac1 = bn_coeffs(s1a, s2a, gm1s, bt1s, "l1")

            # BN + relu -> h1 in one batched ACT op, then fp16 transposes
            # and one bulk store of the normalized layer-2 table
            nc.scalar.activation(
                h1[:], hpre[:], Act.Relu, bias=ac1[:, 1:2], scale=ac1[:, 0:1]
            )
            h1_16 = big.tile([P, T * P], f16, tag="big16a")
            nc.vector.tensor_copy(h1_16[:], h1[:])
            st_all = big.tile([P, T, P], f16, tag="big16b")
            for t in range(T):
                tp = ps.tile([P, P], f16, tag="tp")
                nc.tensor.transpose(
                    out=tp[:],
                    in_=h1_16[:, t * P : (t + 1) * P],
                    identity=ident16[:],
                )
                nc.vector.tensor_scalar(
                    st_all[:, t, :], tp[:], d_out[:, t : t + 1], None, Alu.mult
                )
            nc.sync.dma_start(
                h16_shard.rearrange("(t p) f -> p t f", p=P), st_all[:]
            )
            nc.gpsimd.collective_compute(
                "AllGather",
                Alu.bypass,
                replica_groups=[list(range(N_CORES))],
                ins=[h16_shard[0:SH, :].opt()],
                outs=[h16_full.opt()],
            )

            # ================= layer 2 =================
            s1b = cst.tile([P, T], f32)
            s2b = cst.tile([P, T], f32)
            gconv_layer(h16_full, W2s, s1b, s2b)
            ac2 = bn_coeffs(s1b, s2b, gm2s, bt2s, "l2")

            # batched: h2 = ac2*hpre + c2 (in place), r16 = fp16(relu(h2+h1)),
            # fp16 transposes, then one bulk store of the output rows
            nc.scalar.activation(
                hpre[:], hpre[:], Act.Identity, bias=ac2[:, 1:2], scale=ac2[:, 0:1]
            )
            nc.vector.tensor_tensor(out=hpre[:], in0=hpre[:], in1=h1[:], op=Alu.add)
            r16 = big.tile([P, T * P], f16, tag="big16a")
            nc.vector.tensor_scalar(r16[:], hpre[:], 0.0, None, Alu.max)
            oall = big.tile([P, T, P], f32, tag="bigf32")
            for t in range(T):
                tp = ps.tile([P, P], f16, tag="tp")
                nc.tensor.transpose(
                    out=tp[:],
                    in_=r16[:, t * P : (t + 1) * P],
                    identity=ident16[:],
                )
                nc.vector.tensor_copy(oall[:, t, :], tp[:])
            nc.sync.dma_start(out_t.rearrange("(t p) f -> p t f", p=P), oall[:])

    nc.compile()
    return nc


# ---------------------------------------------------------------------------


_CACHE = {}


def _get_program(meta):
    key = (meta["SH"], meta["T"], meta["CA"], meta["CB"], meta["idx_cols"])
    if key not in _CACHE:
        _CACHE[key] = _build_program(meta)
    return _CACHE[key]


def _build_in_maps(meta, cores, inputs):
    x = np.asarray(inputs["x"], np.float32)
    SH, T = meta["SH"], meta["T"]
    SHP = T * P
    import ml_dtypes
    iota = np.tile(np.arange(P).astype(ml_dtypes.bfloat16), (P, 1))
    in_maps = []
    for k in range(N_CORES):
        c = cores[k]
        xs = np.zeros((SHP, P), np.float32)
        xs[:SH] = x[k * SH : (k + 1) * SH]
        in_maps.append(
            {
                "x_shard": xs,
                "W1": np.asarray(inputs["W1"], np.float32),
                "W2": np.asarray(inputs["W2"], np.float32),
                "gamma1": np.asarray(inputs["gamma1"], np.float32).reshape(P, 1),
                "beta1": np.asarray(inputs["beta1"], np.float32).reshape(P, 1),
                "gamma2": np.asarray(inputs["gamma2"], np.float32).reshape(P, 1),
                "beta2": np.asarray(inputs["beta2"], np.float32).reshape(P, 1),
                "iota": iota,
                "idx_img": c["idx_img"],
                "slotT": c["slotT"],
                "deg_out": c["deg_out"],
                "deg_in": c["deg_in"],
            }
        )
    return in_maps


def kernel(**inputs):
    x = np.asarray(inputs["x"], np.float32)
    src = np.asarray(inputs["src"])
    dst = np.asarray(inputs["dst"])
    n_nodes = x.shape[0]

    meta, cores = _host_prep(src, dst, n_nodes)
    nc = _get_program(meta)
    in_maps = _build_in_maps(meta, cores, inputs)

    from concourse.bass_utils import run_bass_kernel_spmd

    res = run_bass_kernel_spmd(nc, in_maps, core_ids=list(range(N_CORES)))
    SH = meta["SH"]
    out = np.concatenate(
        [res.results[k]["out"][:SH] for k in range(N_CORES)], axis=0
    )
    return out.astype(np.float32)


# revision 32
# speedup vs baseline: 2.3681x; 1.0215x over previous
"""GCN encoder (2-layer, BN, residual) on 8 Trainium2 NeuronCores.

Sharding: nodes partitioned contiguously across 8 cores (6250 each). Edges
bucketed by dst shard on host (integer-only preprocessing: bucket/sort/pad
edge indices, degree counts via bincount). All float math runs on device:

  - per-node norm d_out=rsqrt(clip(outdeg,1)) folded into an fp16 copy of the
    gather table (x*d_out, AllGathered to every core)
  - SpMM: dma_gather of 128-edge chunks (rows->partitions) + one-hot selector
    matmul on PE accumulating m^T[feat, dst] in PSUM; selector built on DVE
    from iota==slot compare (exact 0/1 entries)
  - d_in applied via a broadcast matrix during PSUM evacuation
  - W matmul with W as the stationary operand keeps the [feat, dst] layout so
    BN (per-feature affine) uses per-partition ACT scale/bias + fused ReLU
  - BN stats: per-core partial sums + 1KB AllReduce
  - layer-2 table: h1*d_out cast fp16, AllGathered
"""

import sys

sys.path.insert(0, "/opt/trn_rl_repo")

import numpy as np

P = 128
N_CORES = 8
EPS = 1e-5

# compute dtype for gather tables / selectors / segment matmul
_F16 = "float16"

# dma_gather tuning (device crashes observed for very large single calls)
GATHER_SINGLE_PACKET = False
GATHER_MAX_IDX = 768  # max indices per dma_gather instruction (larger crashes device)
DMA_SCRATCH = 32768  # per-partition SWDGE descriptor-ring carveout
N_SWDGE_QUEUES = 4  # each queue runs on its own Q7 core pair -> parallel desc-gen


def _cdiv(a, b):
    return -(-a // b)


# ---------------------------------------------------------------------------
# host-side integer preprocessing (indices only; no float arithmetic on data)
# ---------------------------------------------------------------------------


def _wrap_idx_image(idx_list):
    """int16 index list (len % 16 == 0) -> [128, len/16] SBUF image.

    dma_gather reads idx i from partition i%16, free slot i//16; the 16-row
    pattern must be replicated 8x across the 128 partitions (one per Q7 core).
    """
    n = idx_list.shape[0]
    assert n % 16 == 0
    img16 = idx_list.reshape(n // 16, 16).T  # [16, n/16]
    return np.tile(img16, (8, 1)).astype(np.int16)  # [128, n/16]


def _host_prep(src, dst, n_nodes):
    """Bucket edges by (dst shard, dst tile, src half); pad to uniform chunk
    capacities so all 8 cores run one identical program."""
    NC = N_CORES
    SH = n_nodes // NC
    assert SH * NC == n_nodes
    T = _cdiv(SH, P)
    SPLIT = n_nodes // 2
    assert SPLIT < 32768 and (n_nodes - SPLIT) <= 32768

    src = np.asarray(src, np.int64)
    dst = np.asarray(dst, np.int64)

    per_core = []
    CA = CB = 1
    for k in range(NC):
        m = (dst >= k * SH) & (dst < (k + 1) * SH)
        s = src[m]
        dl = dst[m] - k * SH
        t_idx = dl // P
        slot = dl % P
        half = (s >= SPLIT).astype(np.int64)
        idxval = np.where(half == 1, s - SPLIT, s)
        per_core.append((t_idx, half, idxval, slot))
        for t in range(T):
            tm = t_idx == t
            na = int(np.count_nonzero(tm & (half == 0)))
            nb = int(np.count_nonzero(tm & (half == 1)))
            CA = max(CA, _cdiv(na, P))
            CB = max(CB, _cdiv(nb, P))

    n_chunks = T * (CA + CB)
    pairs = [(2 * b, min(2 * b + 1, T - 1)) for b in range(_cdiv(T, 2))]

    cores = []
    for k in range(NC):
        t_idx, half, idxval, slot = per_core[k]
        A_idx = np.zeros((T, CA * P), np.int16)
        B_idx = np.zeros((T, CB * P), np.int16)
        # pad slot = 255: one-hot (iota==slot) never fires -> zero column
        slots = np.full((n_chunks, P), 255.0, np.float16)
        for t in range(T):
            tm = t_idx == t
            for h, (Cc, buf) in enumerate(((CA, A_idx), (CB, B_idx))):
                hm = tm & (half == h)
                iv = idxval[hm]
                sl = slot[hm]
                n = iv.shape[0]
                buf[t, :n] = iv.astype(np.int16)
                base = t * (CA + CB) + (0 if h == 0 else CA)
                for c in range(Cc):
                    lo, hi = c * P, min((c + 1) * P, n)
                    if hi > lo:
                        slots[base + c, : hi - lo] = sl[lo:hi].astype(np.float16)

        # gather-call index images: one A call + one B call per tile pair
        imgs = []
        offs_a, offs_b = [], []
        col = 0
        for t0, t1 in pairs:
            tl = [t0] if t0 == t1 else [t0, t1]
            for h, (Cc, buf, offs) in enumerate(
                ((CA, A_idx, offs_a), (CB, B_idx, offs_b))
            ):
                lst = np.concatenate([buf[t] for t in tl])
                img = _wrap_idx_image(lst)
                offs.append((col, img.shape[1], len(tl) * Cc * P))
                col += img.shape[1]
                imgs.append(img)
        idx_img = np.concatenate(imgs, axis=1)  # [128, col]

        # degree counts (integers), tile-column layout [P, T], pad rows deg=1
        outdeg = np.bincount(src, minlength=n_nodes).astype(np.int64)
        indeg = np.bincount(dst, minlength=n_nodes).astype(np.int64)
        mine = slice(k * SH, (k + 1) * SH)

        def _cols(d):
            v = np.ones(T * P, np.float32)
            v[:SH] = d[mine].astype(np.float32)
            return v.reshape(T, P).T.copy()  # [P, T]

        cores.append(
            dict(
                idx_img=idx_img,
                slotT=slots.T.copy(),  # [P, n_chunks] fp16
                deg_out=_cols(outdeg),
                deg_in=_cols(indeg),
                offs_a=offs_a,
                offs_b=offs_b,
            )
        )

    meta = dict(
        SH=SH,
        T=T,
        SPLIT=SPLIT,
        CA=CA,
        CB=CB,
        n_chunks=n_chunks,
        pairs=pairs,
        idx_cols=cores[0]["idx_img"].shape[1],
        n_nodes=n_nodes,
        # call offsets are identical across cores by construction
        offs_a=cores[0]["offs_a"],
        offs_b=cores[0]["offs_b"],
    )
    for c in cores[1:]:
        assert c["offs_a"] == meta["offs_a"] and c["offs_b"] == meta["offs_b"]
        assert c["idx_img"].shape == cores[0]["idx_img"].shape
    return meta, cores


# ---------------------------------------------------------------------------
# device program (identical on all cores; all data-dependence through SBUF)
# ---------------------------------------------------------------------------


def _build_program(meta):
    import concourse.bacc as bacc
    import concourse.bass as bass
    import concourse.tile as tile
    from concourse import mybir
    from concourse.masks import make_identity

    f32 = mybir.dt.float32
    f16 = getattr(mybir.dt, _F16)
    Alu = mybir.AluOpType
    Act = mybir.ActivationFunctionType

    SH, T, SPLIT = meta["SH"], meta["T"], meta["SPLIT"]
    CA, CB = meta["CA"], meta["CB"]
    NCH = meta["n_chunks"]
    NN = meta["n_nodes"]
    pairs = meta["pairs"]
    rows_of = lambda t: min(P, SH - t * P)

    nc = bacc.Bacc(
        "TRN2",
        target_bir_lowering=False,
        debug=False,
        num_devices=N_CORES,
        dynamic_dma_scratch_size=DMA_SCRATCH,
        num_swdge_queues=N_SWDGE_QUEUES,
    )

    # ---- I/O -------------------------------------------------------------
    SHP = T * P  # shard rows padded to a tile multiple
    x_shard = nc.dram_tensor("x_shard", [SHP, P], f32, kind="ExternalInput")
    W1_t = nc.dram_tensor("W1", [P, P], f32, kind="ExternalInput")
    W2_t = nc.dram_tensor("W2", [P, P], f32, kind="ExternalInput")
    gm1 = nc.dram_tensor("gamma1", [P, 1], f32, kind="ExternalInput")
    bt1 = nc.dram_tensor("beta1", [P, 1], f32, kind="ExternalInput")
    gm2 = nc.dram_tensor("gamma2", [P, 1], f32, kind="ExternalInput")
    bt2 = nc.dram_tensor("beta2", [P, 1], f32, kind="ExternalInput")
    iota_t = nc.dram_tensor("iota", [P, P], f16, kind="ExternalInput")
    idx_t = nc.dram_tensor("idx_img", [P, meta["idx_cols"]], mybir.dt.int16,
                           kind="ExternalInput")
    slot_t = nc.dram_tensor("slotT", [P, NCH], f16, kind="ExternalInput")
    dego_t = nc.dram_tensor("deg_out", [P, T], f32, kind="ExternalInput")
    degi_t = nc.dram_tensor("deg_in", [P, T], f32, kind="ExternalInput")
    out_t = nc.dram_tensor("out", [SHP, P], f32, kind="ExternalOutput")

    with tile.TileContext(nc) as tc:
        with (
            tc.tile_pool(name="cst", bufs=1) as cst,
            tc.tile_pool(name="big", bufs=1) as big,
            tc.tile_pool(name="gat", bufs=3) as gat,
            tc.tile_pool(name="wrk", bufs=3) as wrk,
            tc.tile_pool(name="ps", bufs=2, space="PSUM") as ps,
            tc.tile_pool(name="dram", bufs=1, space="DRAM") as dram,
        ):
            # ---- degree normalizers first: the x16 table cast + AllGather
            # is the serial head of the kernel, so issue it before the bulk
            # static-data loads.
            d_out = cst.tile([P, T], f32)
            d_in = cst.tile([P, T], f32)
            for deg_dram, d_sb in ((dego_t, d_out), (degi_t, d_in)):
                raw = wrk.tile([P, T], f32, tag="degraw")
                nc.sync.dma_start(raw[:], deg_dram[:])
                nc.vector.tensor_scalar_max(raw[:], raw[:], 1.0)
                nc.scalar.sqrt(raw[:], raw[:])
                nc.vector.reciprocal(d_sb[:], raw[:])

            # ---- fp16 normalized gather table for layer 1 ---------------
            # bulk load -> one batched scale+cast -> bulk store -> AllGather
            x16_shard = dram.tile([SHP, P], f16)
            x16_full = dram.tile([NN, P], f16, addr_space="Shared")
            xall = big.tile([P, T, P], f32, tag="bigf32")
            nc.sync.dma_start(xall[:], x_shard.rearrange("(t p) f -> p t f", p=P))
            x16all = big.tile([P, T, P], f16, tag="big16a")
            nc.vector.tensor_tensor(
                out=x16all[:],
                in0=xall[:],
                in1=d_out[:, :, None].to_broadcast([P, T, P]),
                op=Alu.mult,
            )
            nc.sync.dma_start(
                x16_shard.rearrange("(t p) f -> p t f", p=P), x16all[:]
            )
            nc.gpsimd.collective_compute(
                "AllGather",
                Alu.bypass,
                replica_groups=[list(range(N_CORES))],
                ins=[x16_shard[0:SH, :].opt()],
                outs=[x16_full.opt()],
            )

            # ---- constants / static data --------------------------------
            ident = cst.tile([P, P], f32)
            make_identity(nc, ident[:])
            W1s = cst.tile([P, P], f32)
            W2s = cst.tile([P, P], f32)
            iota = cst.tile([P, P], f16)
            nc.sync.dma_start(W1s[:], W1_t[:])
            nc.sync.dma_start(W2s[:], W2_t[:])
            nc.sync.dma_start(iota[:], iota_t[:])
            idx_sb = cst.tile([P, meta["idx_cols"]], mybir.dt.int16)
            nc.sync.dma_start(idx_sb[:], idx_t[:])
            slot_sb = cst.tile([P, NCH], f16)
            nc.sync.dma_start(slot_sb[:], slot_t[:])
            gm1s = cst.tile([P, 1], f32)
            bt1s = cst.tile([P, 1], f32)
            gm2s = cst.tile([P, 1], f32)
            bt2s = cst.tile([P, 1], f32)
            nc.sync.dma_start(gm1s[:], gm1[:])
            nc.sync.dma_start(bt1s[:], bt1[:])
            nc.sync.dma_start(gm2s[:], gm2[:])
            nc.sync.dma_start(bt2s[:], bt2[:])
            # fp16 copies of the weight matrices (mT is fp16-derived anyway)
            W1h = cst.tile([P, P], f16)
            W2h = cst.tile([P, P], f16)
            nc.vector.tensor_copy(W1h[:], W1s[:])
            nc.vector.tensor_copy(W2h[:], W2s[:])
            ident16 = cst.tile([P, P], f16)
            nc.vector.tensor_copy(ident16[:], ident[:])

            # d_in broadcast rows: din_bc[:, t*P+j] = d_in[j, t] for all rows
            din_bc = big.tile([P, T * P], f32)
            for t in range(T):
                bc_ps = ps.tile([P, P], f32, tag="tp")
                nc.tensor.transpose(
                    out=bc_ps[:],
                    in_=d_in[:, t : t + 1].to_broadcast([P, P]),
                    identity=ident[:],
                )
                nc.vector.tensor_copy(din_bc[:, t * P : (t + 1) * P], bc_ps[:])

            # persistent stores
            hpre = big.tile([P, T * P], f32)   # pre-BN activations [feat, dst]
            h1 = big.tile([P, T * P], f32)     # post-BN/relu layer-1 output
            h16_shard = dram.tile([SHP, P], f16)
            h16_full = dram.tile([NN, P], f16, addr_space="Shared")

            gq = [0]

            def gconv_layer(table_full, W_sb, s1_cols, s2_cols):
                """SpMM + W matmul; fills hpre and the per-tile stat columns."""
                srcA = table_full[0:SPLIT, :]
                srcB = table_full[SPLIT:NN, :]
                for ip, (t0, t1) in enumerate(pairs):
                    tl = [t0] if t0 == t1 else [t0, t1]
                    bufs = {}
                    for h, (Cc, offs, sv) in enumerate(
                        ((CA, meta["offs_a"], srcA), (CB, meta["offs_b"], srcB))
                    ):
                        col, wcols, nidx = offs[ip]
                        g = gat.tile([P, 2 * Cc, P], f16, tag=f"g{h}")
                        nch = nidx // P
                        step = max(1, GATHER_MAX_IDX // P)
                        for c0 in range(0, nch, step):
                            c1 = min(c0 + step, nch)
                            nc.gpsimd.dma_gather(
                                g[:, c0:c1, :],
                                sv,
                                idx_sb[:, col + c0 * 8 : col + c1 * 8],
                                (c1 - c0) * P,
                                (c1 - c0) * P,
                                P,
                                single_packet=GATHER_SINGLE_PACKET,
                                queue_num=gq[0] % N_SWDGE_QUEUES,
                            )
                            gq[0] += 1
                        bufs[h] = g
                    NCHT = CA + CB
                    mTs_ps = []
                    sels = []
                    for ti, t in enumerate(tl):
                        cid0 = t * NCHT
                        sel = wrk.tile([P, NCHT, P], f16, tag="sel", bufs=4,
                                       name=f"sel{ti}")
                        nc.vector.tensor_tensor(
                            out=sel[:],
                            in0=slot_sb[:, cid0 : cid0 + NCHT][:, :, None]
                            .to_broadcast([P, NCHT, P]),
                            in1=iota[:, None, :].to_broadcast([P, NCHT, P]),
                            op=Alu.is_equal,
                        )
                        sels.append(sel)
                        mTs_ps.append(ps.tile([P, P], f32, tag="mT", bufs=4,
                                              name=f"mT{ti}"))
                    # interleave the two tiles' accumulation chains so the PE
                    # alternates PSUM banks (hides write-commit latency)
                    for c in range(NCHT):
                        h, cc = (0, c) if c < CA else (1, c - CA)
                        Cc = CA if h == 0 else CB
                        for ti in range(len(tl)):
                            nc.tensor.matmul(
                                out=mTs_ps[ti][:],
                                lhsT=bufs[h][:, ti * Cc + cc, :],
                                rhs=sels[ti][:, c, :],
                                start=(c == 0),
                                stop=(c == NCHT - 1),
                            )
                    for ti, t in enumerate(tl):
                        # evacuate with d_in column scaling (fp16 for the W mm)
                        mTs = wrk.tile([P, P], f16, tag="mTs")
                        nc.vector.tensor_tensor(
                            out=mTs[:],
                            in0=mTs_ps[ti][:],
                            in1=din_bc[:, t * P : (t + 1) * P],
                            op=Alu.mult,
                        )
                        hp = ps.tile([P, P], f32, tag="hp")
                        nc.tensor.matmul(
                            out=hp[:], lhsT=W_sb[:], rhs=mTs[:], start=True, stop=True
                        )
                        # evacuate + per-feature partial sums for BN
                        nc.vector.tensor_scalar(
                            hpre[:, t * P : (t + 1) * P],
                            hp[:],
                            1.0,
                            None,
                            Alu.mult,
                            Alu.add,
                            accum_out=s1_cols[:, t : t + 1],
                        )
                        sq = wrk.tile([P, P], f16, tag="sq")
                        nc.scalar.activation(
                            sq[:],
                            hpre[:, t * P : (t + 1) * P],
                            Act.Square,
                            accum_out=s2_cols[:, t : t + 1],
                        )

            def bn_coeffs(s1_cols, s2_cols, gam, bet, tag):
                """AllReduce partial sums -> per-feature scale a, shift c."""
                stats_in = dram.tile([P, 2], f32, name=f"stats_in_{tag}")
                stats_out = dram.tile(
                    [P, 2], f32, addr_space="Shared", name=f"stats_out_{tag}"
                )
                pack = wrk.tile([P, 2], f32, tag="pack")
                nc.vector.tensor_reduce(
                    pack[:, 0:1], s1_cols[:], axis=mybir.AxisListType.X, op=Alu.add
                )
                nc.vector.tensor_reduce(
                    pack[:, 1:2], s2_cols[:], axis=mybir.AxisListType.X, op=Alu.add
                )
                nc.sync.dma_start(stats_in[:], pack[:])
                nc.gpsimd.collective_compute(
                    "AllReduce",
                    Alu.add,
                    replica_groups=[list(range(N_CORES))],
                    ins=[stats_in.opt()],
                    outs=[stats_out.opt()],
                )
                glob = wrk.tile([P, 2], f32, tag="glob")
                nc.sync.dma_start(glob[:], stats_out[:])
                mo = wrk.tile([P, 4], f32, tag="mo")
                # mo: 0=mu 1=E[h^2] 2=var+eps 3=scratch
                nc.vector.tensor_scalar(mo[:, 0:2], glob[:], 1.0 / NN, None, Alu.mult)
                nc.vector.tensor_tensor(
                    out=mo[:, 3:4], in0=mo[:, 0:1], in1=mo[:, 0:1], op=Alu.mult
                )
                nc.vector.tensor_tensor(
                    out=mo[:, 2:3], in0=mo[:, 1:2], in1=mo[:, 3:4], op=Alu.subtract
                )
                nc.vector.tensor_scalar_add(mo[:, 2:3], mo[:, 2:3], EPS)
                nc.scalar.sqrt(mo[:, 2:3], mo[:, 2:3])
                a_c = cst.tile([P, 2], f32, name=f"a_c_{gam.name}")
                nc.vector.reciprocal(a_c[:, 0:1], mo[:, 2:3])
                nc.vector.tensor_tensor(
                    out=a_c[:, 0:1], in0=a_c[:, 0:1], in1=gam[:], op=Alu.mult
                )
                nc.vector.tensor_tensor(
                    out=a_c[:, 1:2], in0=a_c[:, 0:1], in1=mo[:, 0:1], op=Alu.mult
                )
                nc.vector.tensor_tensor(
                    out=a_c[:, 1:2], in0=bet[:], in1=a_c[:, 1:2], op=Alu.subtract
                )
                return a_c

            # ================= layer 1 =================
            s1a = cst.tile([P, T], f32)
            s2a = cst.tile([P, T], f32)
            gconv_layer(x16_full, W1h, s1a, s2a)
            ac1 = bn_coeffs(s1a, s2a, gm1s, bt1s, "l1")

            # BN + relu -> h1 in one batched ACT op, then fp16 transposes
            # and one bulk store of the normalized layer-2 table
            nc.scalar.activation(
                h1[:], hpre[:], Act.Relu, bias=ac1[:, 1:2], scale=ac1[:, 0:1]
            )
            h1_16 = big.tile([P, T * P], f16, tag="big16a")
            nc.vector.tensor_copy(h1_16[:], h1[:])
            st_all = big.tile([P, T, P], f16, tag="big16b")
            for t in range(T):
                tp = ps.tile([P, P], f16, tag="tp")
                nc.tensor.transpose(
                    out=tp[:],
                    in_=h1_16[:, t * P : (t + 1) * P],
                    identity=ident16[:],
                )
                nc.vector.tensor_scalar(
                    st_all[:, t, :], tp[:], d_out[:, t : t + 1], None, Alu.mult
                )
            nc.sync.dma_start(
                h16_shard.rearrange("(t p) f -> p t f", p=P), st_all[:]
            )
            nc.gpsimd.collective_compute(
                "AllGather",
                Alu.bypass,
                replica_groups=[list(range(N_CORES))],
                ins=[h16_shard[0:SH, :].opt()],
                outs=[h16_full.opt()],
            )

            # ================= layer 2 =================
            s1b = cst.tile([P, T], f32)
            s2b = cst.tile([P, T], f32)
            gconv_layer(h16_full, W2s, s1b, s2b)
            ac2 = bn_coeffs(s1b, s2b, gm2s, bt2s, "l2")

            # batched: h2 = ac2*hpre + c2 (in place), r16 = fp16(relu(h2+h1)),
            # fp16 transposes, then one bulk store of the output rows
            nc.scalar.activation(
                hpre[:], hpre[:], Act.Identity, bias=ac2[:, 1:2], scale=ac2[:, 0:1]
            )
            nc.vector.tensor_tensor(out=hpre[:], in0=hpre[:], in1=h1[:], op=Alu.add)
            r16 = big.tile([P, T * P], f16, tag="big16a")
            nc.vector.tensor_scalar(r16[:], hpre[:], 0.0, None, Alu.max)
            oall = big.tile([P, T, P], f32, tag="bigf32")
            for t in range(T):
                tp = ps.tile([P, P], f16, tag="tp")
                nc.tensor.transpose(
                    out=tp[:],
                    in_=r16[:, t * P : (t + 1) * P],
                    identity=ident16[:],
                )
                nc.vector.tensor_copy(oall[:, t, :], tp[:])
            nc.sync.dma_start(out_t.rearrange("(t p) f -> p t f", p=P), oall[:])

    nc.compile()
    return nc


# ---------------------------------------------------------------------------


_CACHE = {}


def _get_program(meta):
    key = (meta["SH"], meta["T"], meta["CA"], meta["CB"], meta["idx_cols"])
    if key not in _CACHE:
        _CACHE[key] = _build_program(meta)
    return _CACHE[key]


def _build_in_maps(meta, cores, inputs):
    x = np.asarray(inputs["x"], np.float32)
    SH, T = meta["SH"], meta["T"]
    SHP = T * P
    iota = np.tile(np.arange(P, dtype=np.float16), (P, 1))
    in_maps = []
    for k in range(N_CORES):
        c = cores[k]
        xs = np.zeros((SHP, P), np.float32)
        xs[:SH] = x[k * SH : (k + 1) * SH]
        in_maps.append(
            {
                "x_shard": xs,
                "W1": np.asarray(inputs["W1"], np.float32),
                "W2": np.asarray(inputs["W2"], np.float32),
                "gamma1": np.asarray(inputs["gamma1"], np.float32).reshape(P, 1),
                "beta1": np.asarray(inputs["beta1"], np.float32).reshape(P, 1),
                "gamma2": np.asarray(inputs["gamma2"], np.float32).reshape(P, 1),
                "beta2": np.asarray(inputs["beta2"], np.float32).reshape(P, 1),
                "iota": iota,
                "idx_img": c["idx_img"],
                "slotT": c["slotT"],
                "deg_out": c["deg_out"],
                "deg_in": c["deg_in"],
            }
        )
    return in_maps


def kernel(**inputs):
    x = np.asarray(inputs["x"], np.float32)
    src = np.asarray(inputs["src"])
    dst = np.asarray(inputs["dst"])
    n_nodes = x.shape[0]

    meta, cores = _host_prep(src, dst, n_nodes)
    nc = _get_program(meta)
    in_maps = _build_in_maps(meta, cores, inputs)

    from concourse.bass_utils import run_bass_kernel_spmd

    res = run_bass_kernel_spmd(nc, in_maps, core_ids=list(range(N_CORES)))
    SH = meta["SH"]
    out = np.concatenate(
        [res.results[k]["out"][:SH] for k in range(N_CORES)], axis=0
    )
    return out.astype(np.float32)
